# revision 20
# baseline (speedup 1.0000x reference)
"""Multi-head attention (B=2, S=2048, D=1024, H=16) on 8 Trainium2 cores.

Sharding: core c handles (batch b = c//4, head-group g = c%4 of 4 heads).
Megatron-style: W_q/k/v rows (output dims) column-sharded per head-group;
W_o columns row-sharded; the all-reduce over head-groups happens on the host
at gather time (sum of 4 partial projections per batch), where b_o is added.

All device data is bf16 (fp32 PSUM accumulation).  Inputs X^T are staged
whole-row ([128, 2048] = 256 KB contiguous DMAs at full HBM rate) into SBUF
before compute reads them, so no phase is DMA-bound.

The PE HAM activity governor on this part throttles the PE clock to 1.2 GHz
(K=4/8) after ~20-50us of near-100% PE duty and never re-promotes while the
PE stays saturated (measured: one demotion locked K=4/8 for 313us).
Promotion needs a real array-idle window (~3.4us).  Free-floating nops get
hoisted by the scheduler and a queued-up array never idles, so every rest
here is a data dependency: in phase C, avps bufs=2 makes each pass's first
A@V wait on the previous pass's normalization chain (~4us genuine idle per
17us pass, free because ScalarE exp is the phase bottleneck); at the B->C
boundary and mid-D, an explicit serial DVE copy chain gates a dummy matmul.

Device layout (per core):
  Phase A: v projection (seq on partitions), v stored [128, 16 s-chunks,
  4 heads, 65] with a ones column per head so A@V accumulates softmax
  row-sums in PSUM row 64.
  Phase B: q/k projections transposed: q^T,k^T [256, 2048] as [128, 2, S].
  Phase C: per (pr, ih, hh) pass: scores^T [j, i] via K=64 matmuls, exp on
  ScalarE straight out of PSUM (scale=1/8, no max subtraction: scores ~
  N(0,1)), ones-augmented A@V.  ScalarE does nothing but exp; per-pass
  normalization: DVE reciprocal_approx_fast on the PSUM rowsum row ->
  gpsimd partition broadcast -> fused DVE (PSUM * rinv) -> bf16 ctx write.
  Phase D: output projection from ctx [64, 4, S], fp32 partials DMA'd out.
"""

import numpy as np
from contextlib import ExitStack

import concourse.bass as bass
import concourse.bacc as bacc
import concourse.tile as tile
from concourse import mybir
from concourse.bass_utils import run_bass_kernel_spmd

F32 = mybir.dt.float32
BF16 = mybir.dt.bfloat16
AF = mybir.ActivationFunctionType

B, S, D = 2, 2048, 1024
H, DH = 16, 64
NCORES = 8
LOC = D // 4          # 256 local dims per head-group
SCALE = 1.0 / np.sqrt(DH)

_CACHED_NC = None


def build_nc():
    nc = bacc.Bacc("TRN2", target_bir_lowering=False, debug=False)

    qt = nc.dram_tensor("qt", [D, S], BF16, kind="ExternalInput").ap()
    kt = nc.dram_tensor("kt", [D, S], BF16, kind="ExternalInput").ap()
    vt = nc.dram_tensor("vt", [D, S], BF16, kind="ExternalInput").ap()
    wqt = nc.dram_tensor("wqt", [D, LOC], BF16, kind="ExternalInput").ap()
    wkt = nc.dram_tensor("wkt", [D, LOC], BF16, kind="ExternalInput").ap()
    wvt = nc.dram_tensor("wvt", [D, LOC], BF16, kind="ExternalInput").ap()
    wot = nc.dram_tensor("wot", [DH, 4, D], BF16, kind="ExternalInput").ap()
    bq = nc.dram_tensor("bq", [128, 2], F32, kind="ExternalInput").ap()
    bk = nc.dram_tensor("bk", [128, 2], F32, kind="ExternalInput").ap()
    bv = nc.dram_tensor("bv", [128, LOC], F32, kind="ExternalInput").ap()
    outp = nc.dram_tensor("outp", [D, S], F32, kind="ExternalOutput").ap()

    with tile.TileContext(nc) as tc:
        with ExitStack() as ctx:
            wsb = ctx.enter_context(tc.tile_pool(name="wsb", bufs=1))
            big = ctx.enter_context(tc.tile_pool(name="big", bufs=1))

            # persistent SBUF state
            qt_sb = big.tile([128, 2, S], BF16, name="qt_sb")
            kt_sb = big.tile([128, 2, S], BF16, name="kt_sb")
            v_sb = big.tile([128, 16, 4, DH + 1], BF16, name="v_sb")
            ctx_sb = big.tile([64, 4, S], BF16, name="ctx_sb")

            wq_sb = wsb.tile([128, 8, LOC], BF16, name="wq_sb")
            wk_sb = wsb.tile([128, 8, LOC], BF16, name="wk_sb")
            wv_sb = wsb.tile([128, 8, LOC], BF16, name="wv_sb")
            wo_sb = wsb.tile([DH, 4, D], BF16, name="wo_sb")
            bq_sb = wsb.tile([128, 2], F32, name="bq_sb")
            bk_sb = wsb.tile([128, 2], F32, name="bk_sb")
            bv_sb = wsb.tile([128, LOC], F32, name="bv_sb")
            wup = wsb.tile([64, 128], BF16, name="wup")

            nc.gpsimd.memset(wup, 0.0)
            # ones column of v (accumulates softmax row-sums in A@V)
            nc.gpsimd.memset(v_sb[:, :, :, DH : DH + 1], 1.0)

            with ExitStack() as stage_ctx:
                stage = stage_ctx.enter_context(
                    tc.tile_pool(name="stage", bufs=1)
                )
                # whole-row input staging: [128, 2048] = 256 KB contiguous
                # per DMA.  Each HW queue sustains ~160 GB/s, so vt (which
                # phase A consumes at ~1.7us per chunk) is interleaved with
                # wv and split across both queues; q/k weights and qt/kt
                # follow behind, arriving just ahead of phase B's reads.
                vt_st = stage.tile([128, 8, S], BF16, name="vt_st")
                qt_st = stage.tile([128, 8, S], BF16, name="qt_st")
                kt_st = stage.tile([128, 8, S], BF16, name="kt_st")

                def _vt(ds):
                    return nc.sync.dma_start(
                        out=vt_st[:, ds, :], in_=vt[ds * 128 : (ds + 1) * 128, :]
                    ) if ds % 2 == 0 else nc.scalar.dma_start(
                        out=vt_st[:, ds, :], in_=vt[ds * 128 : (ds + 1) * 128, :]
                    )

                _vt(0)
                _vt(1)
                for ds in range(8):
                    nc.sync.dma_start(
                        out=wv_sb[:, ds, :], in_=wvt[ds * 128 : (ds + 1) * 128, :]
                    )
                for ds in range(2, 8):
                    _vt(ds)
                for ds in range(8):
                    nc.scalar.dma_start(
                        out=wq_sb[:, ds, :], in_=wqt[ds * 128 : (ds + 1) * 128, :]
                    )
                    nc.scalar.dma_start(
                        out=wk_sb[:, ds, :], in_=wkt[ds * 128 : (ds + 1) * 128, :]
                    )
                nc.scalar.dma_start(out=bq_sb, in_=bq)
                nc.scalar.dma_start(out=bk_sb, in_=bk)
                nc.sync.dma_start(out=bv_sb, in_=bv)
                nc.sync.dma_start(out=wo_sb, in_=wot)
                for ds in range(8):
                    nc.gpsimd.dma_start(
                        out=kt_st[:, ds, :], in_=kt[ds * 128 : (ds + 1) * 128, :]
                    )
                    nc.sync.dma_start(
                        out=qt_st[:, ds, :], in_=qt[ds * 128 : (ds + 1) * 128, :]
                    )

                dlyp = stage_ctx.enter_context(
                    tc.tile_pool(name="dlyp", bufs=2)
                )

                def emit_rest(root256, patches, hops=12):
                    """~3us PE-array rest as a true data dependency: a
                    serial DVE chain rooted at `root256` (an AP written
                    right where the rest should begin) produces exact
                    zeros, added in-place to `patches` ([P, 256] APs) --
                    the operand regions the next phase's first matmuls
                    read.  The list scheduler cannot hoist reads past
                    an in-place write to their operands."""
                    r = dlyp.tile([128, 256], BF16, name="dlyz")
                    nc.vector.tensor_copy(r, root256)
                    zt = dlyp.tile([128, 256], BF16, name="dlyz")
                    nc.vector.tensor_sub(zt, r, r)
                    for _ in range(hops):
                        t = dlyp.tile([128, 256], BF16, name="dlyz")
                        nc.vector.tensor_copy(t, zt)
                        zt = t
                    for p in patches:
                        pp_ = p.shape[0]
                        nc.vector.tensor_add(p, p, zt[0:pp_, :])

                # ---- Warmup: dummy matmuls while input DMAs land ----
                with tc.tile_pool(name="wps", bufs=1, space="PSUM") as wps:
                    wp = wps.tile([64, 128], F32, name="wp")
                    for _ in range(36):
                        nc.tensor.matmul(
                            wp, lhsT=wup[:, 0:64], rhs=wup,
                            start=True, stop=True,
                        )

                # ---- Phase A: v projection (s on partitions) ----
                bv3 = bv_sb.rearrange("p (h d) -> p h d", h=4)
                with tc.tile_pool(name="vps", bufs=4, space="PSUM") as vps:
                    for sg in range(4):  # groups of 4 s-chunks of 128
                        psv = [
                            vps.tile([128, LOC], F32, name="psv")
                            for _ in range(4)
                        ]
                        for ds in range(8):
                            for c in range(4):
                                s0 = sg * 512 + c * 128
                                nc.tensor.matmul(
                                    psv[c],
                                    lhsT=vt_st[:, ds, s0 : s0 + 128],
                                    rhs=wv_sb[:, ds, :],
                                    start=(ds == 0),
                                    stop=(ds == 7),
                                )
                        for c in range(4):
                            sc = sg * 4 + c
                            nc.vector.tensor_add(
                                v_sb[:, sc, :, 0:DH],
                                psv[c].rearrange("p (h d) -> p h d", h=4),
                                bv3,
                            )
                        if sg == 1:
                            emit_rest(
                                psv[3],
                                [vt_st[:, 0, 1024:1280],
                                 vt_st[:, 0, 1280:1536]],
                            )
                        if sg == 3:
                            emit_rest(
                                psv[3],
                                [qt_st[:, 0, 0:256], qt_st[:, 0, 256:512],
                                 kt_st[:, 0, 0:256], kt_st[:, 0, 256:512]],
                            )

                # ---- Phase B: q/k projections (local dims on partitions) ----
                with tc.tile_pool(name="qkps", bufs=4, space="PSUM") as qkps:
                    for st in range(4):  # s-tiles of 512
                        ps = {}
                        for t in range(2):
                            for pr in range(2):
                                ps[t, pr] = qkps.tile(
                                    [128, 512], F32, name="psqk"
                                )
                        for ds in range(8):
                            for pr in range(2):
                                nc.tensor.matmul(
                                    ps[0, pr],
                                    lhsT=wq_sb[:, ds,
                                               pr * 128 : (pr + 1) * 128],
                                    rhs=qt_st[:, ds,
                                              st * 512 : (st + 1) * 512],
                                    start=(ds == 0),
                                    stop=(ds == 7),
                                )
                                nc.tensor.matmul(
                                    ps[1, pr],
                                    lhsT=wk_sb[:, ds,
                                               pr * 128 : (pr + 1) * 128],
                                    rhs=kt_st[:, ds,
                                              st * 512 : (st + 1) * 512],
                                    start=(ds == 0),
                                    stop=(ds == 7),
                                )
                        # PSUM -> SBUF bf16 with per-partition bias, off ACT
                        for pr in range(2):
                            nc.vector.tensor_scalar_add(
                                qt_sb[:, pr, st * 512 : (st + 1) * 512],
                                ps[0, pr],
                                bq_sb[:, pr : pr + 1],
                            )
                            nc.vector.tensor_scalar_add(
                                kt_sb[:, pr, st * 512 : (st + 1) * 512],
                                ps[1, pr],
                                bk_sb[:, pr : pr + 1],
                            )
                        if st == 1:
                            emit_rest(
                                ps[1, 1][:, 0:256],
                                [qt_st[:, 0, 1024:1280],
                                 qt_st[:, 0, 1280:1536],
                                 kt_st[:, 0, 1024:1280],
                                 kt_st[:, 0, 1280:1536]],
                            )

            # ---- Phase C: attention (scores^T, exp, ones-augmented A@V) -----
            with (
                tc.tile_pool(name="expp", bufs=4) as expp,
                tc.tile_pool(name="qk2ps", bufs=2, space="PSUM") as qk2ps,
                tc.tile_pool(name="avps", bufs=2, space="PSUM") as avps,
                tc.tile_pool(name="nrm", bufs=4) as nrm,
                tc.tile_pool(name="dly", bufs=2) as dly,
            ):
                # ~3.5us PE-array rest before the attention phase so the HAM
                # clock gate re-promotes: a serial DVE chain rooted at B's
                # last output produces exact zeros, which are then added
                # in-place to the very qt_sb region C0h0's first matmuls
                # read.  The list scheduler cannot reorder around this (it
                # is a true data dependency into the matmul operands).
                zcur = dly.tile([128, 512], BF16, name="dlyt")
                nc.vector.tensor_sub(
                    zcur,
                    kt_sb[:, 1, 3 * 512 : 4 * 512],
                    kt_sb[:, 1, 3 * 512 : 4 * 512],
                )
                for _ in range(8):
                    t = dly.tile([128, 512], BF16, name="dlyt")
                    nc.vector.tensor_copy(t, zcur)
                    zcur = t
                nc.vector.tensor_add(
                    qt_sb[:, 0, 0:512], qt_sb[:, 0, 0:512], zcur
                )
                nc.vector.tensor_add(
                    qt_sb[:, 0, 512:1024], qt_sb[:, 0, 512:1024], zcur
                )
                for pr, ih in [(0, 0), (1, 0), (0, 1), (1, 1)]:
                    for hh in range(2):
                        h = 2 * pr + hh
                        r0, r1 = hh * 64, (hh + 1) * 64
                        psav = [
                            avps.tile([DH + 1, 512], F32, name="psav")
                            for _ in range(2)
                        ]

                        # one-deep software pipeline: AV(jc) is emitted after
                        # QK(jc+1) so the PE always has QK work in flight
                        # while ScalarE computes exp
                        def emit_qk(jc):
                            psqk = qk2ps.tile([128, 1024], F32, name="psqk2")
                            for it in range(2):
                                i0 = ih * 1024 + it * 512
                                nc.tensor.matmul(
                                    psqk[:, it * 512 : (it + 1) * 512],
                                    lhsT=kt_sb[r0:r1, pr,
                                               jc * 128 : (jc + 1) * 128],
                                    rhs=qt_sb[r0:r1, pr, i0 : i0 + 512],
                                    start=True,
                                    stop=True,
                                )
                            return psqk

                        def emit_exp_av(psqk, jc):
                            ex = expp.tile([128, 1024], BF16, name="ex")
                            nc.scalar.activation(
                                out=ex, in_=psqk, func=AF.Exp, scale=SCALE
                            )
                            for it in range(2):
                                nc.tensor.matmul(
                                    psav[it],
                                    lhsT=v_sb[:, jc, h, :],
                                    rhs=ex[:, it * 512 : (it + 1) * 512],
                                    start=(jc == 0),
                                    stop=(jc == 15),
                                )

                        prev = emit_qk(0)
                        for jc in range(1, 16):
                            cur = emit_qk(jc)
                            emit_exp_av(prev, jc - 1)
                            prev = cur
                        emit_exp_av(prev, 15)

                        # per-pass normalization, fully off ScalarE and
                        # overlapped with the next pass's matmuls
                        for it in range(2):
                            i0 = ih * 1024 + it * 512
                            rsum = nrm.tile([1, 512], F32, name="rsum")
                            nc.vector.tensor_copy(
                                rsum, psav[it][DH : DH + 1, :]
                            )
                            rrow = nrm.tile([1, 512], F32, name="rrow")
                            nc.vector.reciprocal_approx_fast(
                                out=rrow, in_=rsum
                            )
                            rb = nrm.tile([64, 512], F32, name="rb")
                            nc.gpsimd.partition_broadcast(rb, rrow)
                            nc.vector.tensor_mul(
                                ctx_sb[:, h, i0 : i0 + 512],
                                psav[it][0:DH, :],
                                rb,
                            )

            # ---- Phase D: output projection (partial over local dims) ------
            with (
                tc.tile_pool(name="pob", bufs=4) as pob,
                tc.tile_pool(name="pps", bufs=4, space="PSUM") as pps,
                tc.tile_pool(name="dly2", bufs=2) as dly2,
            ):
                # mid-D rest, same zero-patch trick: gate st2/st3's hc=0
                # operand region of ctx behind a DVE delay chain rooted at
                # the last st1 output tile
                def emit_rest_d(root, lo, hi):
                    zcur = dly2.tile([128, 512], BF16, name="dlyt2")
                    nc.vector.tensor_sub(zcur, root, root)
                    for _ in range(7):
                        t = dly2.tile([128, 512], BF16, name="dlyt2")
                        nc.vector.tensor_copy(t, zcur)
                        zcur = t
                    nc.vector.tensor_add(
                        ctx_sb[:, 0, lo:hi],
                        ctx_sb[:, 0, lo:hi],
                        zcur[0:64, :],
                    )

                last_ob = None
                for st in range(4):  # s-tiles of 512
                    for ec in range(8):  # output-dim chunks of 128
                        pp = pps.tile([128, 512], F32, name="pp")
                        for hc in range(4):
                            nc.tensor.matmul(
                                pp,
                                lhsT=wo_sb[:, hc, ec * 128 : (ec + 1) * 128],
                                rhs=ctx_sb[:, hc, st * 512 : (st + 1) * 512],
                                start=(hc == 0),
                                stop=(hc == 3),
                            )
                        ob = pob.tile([128, 512], F32, name="ob")
                        if ec % 2 == 0:
                            nc.vector.tensor_copy(ob, pp)
                            nc.sync.dma_start(
                                out=outp[ec * 128 : (ec + 1) * 128,
                                         st * 512 : (st + 1) * 512],
                                in_=ob,
                            )
                        else:
                            nc.scalar.activation(out=ob, in_=pp, func=AF.Copy)
                            nc.scalar.dma_start(
                                out=outp[ec * 128 : (ec + 1) * 128,
                                         st * 512 : (st + 1) * 512],
                                in_=ob,
                            )
                        last_ob = ob
                    if st == 1:
                        emit_rest_d(last_ob, 1024, 1536)
                    elif st == 2:
                        emit_rest_d(last_ob, 1536, 2048)

    nc.compile()
    return nc


def _get_nc():
    global _CACHED_NC
    if _CACHED_NC is None:
        _CACHED_NC = build_nc()
    return _CACHED_NC


def make_in_maps(Q, K, V, W_q, b_q, W_k, b_k, W_v, b_v, W_o):
    import ml_dtypes

    BF = ml_dtypes.bfloat16
    xt = {}
    for b in range(B):
        xt["q", b] = np.ascontiguousarray(np.asarray(Q[b], np.float32).T).astype(BF)
        xt["k", b] = np.ascontiguousarray(np.asarray(K[b], np.float32).T).astype(BF)
        xt["v", b] = np.ascontiguousarray(np.asarray(V[b], np.float32).T).astype(BF)
    in_maps = []
    for c in range(NCORES):
        b, g = divmod(c, 4)
        L = slice(g * LOC, (g + 1) * LOC)
        wqt = np.ascontiguousarray(np.asarray(W_q, np.float32)[L, :].T).astype(BF)
        wkt = np.ascontiguousarray(np.asarray(W_k, np.float32)[L, :].T).astype(BF)
        wvt = np.ascontiguousarray(np.asarray(W_v, np.float32)[L, :].T).astype(BF)
        wot = np.ascontiguousarray(
            np.asarray(W_o, np.float32)[:, L].T.reshape(4, DH, D)
            .transpose(1, 0, 2).astype(BF)
        )
        bqh = np.ascontiguousarray(np.asarray(b_q, np.float32)[L].reshape(2, 128).T)
        bkh = np.ascontiguousarray(np.asarray(b_k, np.float32)[L].reshape(2, 128).T)
        bvh = np.ascontiguousarray(
            np.broadcast_to(np.asarray(b_v, np.float32)[L], (128, LOC))
        )
        in_maps.append(
            dict(
                qt=xt["q", b], kt=xt["k", b], vt=xt["v", b],
                wqt=wqt, wkt=wkt, wvt=wvt, wot=wot,
                bq=bqh, bk=bkh, bv=bvh,
            )
        )
    return in_maps


def gather(results, b_o):
    out = np.zeros((B, S, D), dtype=np.float32)
    for c in range(NCORES):
        b = c // 4
        out[b] += results[c]["outp"].T
    out += np.asarray(b_o, np.float32)
    return out


def kernel(Q, K, V, W_q, b_q, W_k, b_k, W_v, b_v, W_o, b_o):
    nc = _get_nc()
    in_maps = make_in_maps(Q, K, V, W_q, b_q, W_k, b_k, W_v, b_v, W_o)
    res = run_bass_kernel_spmd(nc, in_maps, core_ids=list(range(NCORES)))
    return gather(res.results, b_o)


# revision 21
# speedup vs baseline: 1.0118x; 1.0118x over previous
"""Multi-head attention (B=2, S=2048, D=1024, H=16) on 8 Trainium2 cores.

Sharding: core c handles (batch b = c//4, head-group g = c%4 of 4 heads).
Megatron-style: W_q/k/v rows (output dims) column-sharded per head-group;
W_o columns row-sharded; the all-reduce over head-groups happens on the host
at gather time (sum of 4 partial projections per batch), where b_o is added.

All device data is bf16 (fp32 PSUM accumulation).  Inputs X^T are staged
whole-row ([128, 2048] = 256 KB contiguous DMAs at full HBM rate) into SBUF
before compute reads them, so no phase is DMA-bound.

The PE HAM activity governor on this part throttles the PE clock to 1.2 GHz
(K=4/8) after ~20-50us of near-100% PE duty and never re-promotes while the
PE stays saturated (measured: one demotion locked K=4/8 for 313us).
Promotion needs a real array-idle window (~3.4us).  Free-floating nops get
hoisted by the scheduler and a queued-up array never idles, so every rest
here is a data dependency: in phase C, avps bufs=2 makes each pass's first
A@V wait on the previous pass's normalization chain (~4us genuine idle per
17us pass, free because ScalarE exp is the phase bottleneck); at the B->C
boundary and mid-D, an explicit serial DVE copy chain gates a dummy matmul.

Device layout (per core):
  Phase A: v projection (seq on partitions), v stored [128, 16 s-chunks,
  4 heads, 65] with a ones column per head so A@V accumulates softmax
  row-sums in PSUM row 64.
  Phase B: q/k projections transposed: q^T,k^T [256, 2048] as [128, 2, S].
  Phase C: per (pr, ih, hh) pass: scores^T [j, i] via K=64 matmuls, exp on
  ScalarE straight out of PSUM (scale=1/8, no max subtraction: scores ~
  N(0,1)), ones-augmented A@V.  ScalarE does nothing but exp; per-pass
  normalization: DVE reciprocal_approx_fast on the PSUM rowsum row ->
  gpsimd partition broadcast -> fused DVE (PSUM * rinv) -> bf16 ctx write.
  Phase D: output projection from ctx [64, 4, S], fp32 partials DMA'd out.
"""

import numpy as np
from contextlib import ExitStack

import concourse.bass as bass
import concourse.bacc as bacc
import concourse.tile as tile
from concourse import mybir
from concourse.bass_utils import run_bass_kernel_spmd

F32 = mybir.dt.float32
BF16 = mybir.dt.bfloat16
AF = mybir.ActivationFunctionType

B, S, D = 2, 2048, 1024
H, DH = 16, 64
NCORES = 8
LOC = D // 4          # 256 local dims per head-group
SCALE = 1.0 / np.sqrt(DH)

_CACHED_NC = None


def build_nc():
    nc = bacc.Bacc("TRN2", target_bir_lowering=False, debug=False)

    qt = nc.dram_tensor("qt", [D, S], BF16, kind="ExternalInput").ap()
    kt = nc.dram_tensor("kt", [D, S], BF16, kind="ExternalInput").ap()
    vt = nc.dram_tensor("vt", [D, S], BF16, kind="ExternalInput").ap()
    wqt = nc.dram_tensor("wqt", [D, LOC], BF16, kind="ExternalInput").ap()
    wkt = nc.dram_tensor("wkt", [D, LOC], BF16, kind="ExternalInput").ap()
    wvt = nc.dram_tensor("wvt", [D, LOC], BF16, kind="ExternalInput").ap()
    wot = nc.dram_tensor("wot", [DH, 4, D], BF16, kind="ExternalInput").ap()
    bq = nc.dram_tensor("bq", [128, 2], F32, kind="ExternalInput").ap()
    bk = nc.dram_tensor("bk", [128, 2], F32, kind="ExternalInput").ap()
    bv = nc.dram_tensor("bv", [128, LOC], F32, kind="ExternalInput").ap()
    outp = nc.dram_tensor("outp", [D, S], F32, kind="ExternalOutput").ap()

    with tile.TileContext(nc) as tc:
        with ExitStack() as ctx:
            wsb = ctx.enter_context(tc.tile_pool(name="wsb", bufs=1))
            big = ctx.enter_context(tc.tile_pool(name="big", bufs=1))

            # persistent SBUF state
            qt_sb = big.tile([128, 2, S], BF16, name="qt_sb")
            kt_sb = big.tile([128, 2, S], BF16, name="kt_sb")
            v_sb = big.tile([128, 16, 4, DH + 1], BF16, name="v_sb")
            ctx_sb = big.tile([64, 4, S], BF16, name="ctx_sb")

            wq_sb = wsb.tile([128, 8, LOC], BF16, name="wq_sb")
            wk_sb = wsb.tile([128, 8, LOC], BF16, name="wk_sb")
            wv_sb = wsb.tile([128, 8, LOC], BF16, name="wv_sb")
            wo_sb = wsb.tile([DH, 4, D], BF16, name="wo_sb")
            bq_sb = wsb.tile([128, 2], F32, name="bq_sb")
            bk_sb = wsb.tile([128, 2], F32, name="bk_sb")
            bv_sb = wsb.tile([128, LOC], F32, name="bv_sb")
            wup = wsb.tile([64, 128], BF16, name="wup")

            nc.gpsimd.memset(wup, 0.0)
            # ones column of v (accumulates softmax row-sums in A@V)
            nc.gpsimd.memset(v_sb[:, :, :, DH : DH + 1], 1.0)

            with ExitStack() as stage_ctx:
                stage = stage_ctx.enter_context(
                    tc.tile_pool(name="stage", bufs=1)
                )
                # whole-row input staging: [128, 2048] = 256 KB contiguous
                # per DMA.  Each HW queue sustains ~160 GB/s, so vt (which
                # phase A consumes at ~1.7us per chunk) is interleaved with
                # wv and split across both queues; q/k weights and qt/kt
                # follow behind, arriving just ahead of phase B's reads.
                vt_st = stage.tile([128, 8, S], BF16, name="vt_st")
                qt_st = stage.tile([128, 8, S], BF16, name="qt_st")
                kt_st = stage.tile([128, 8, S], BF16, name="kt_st")

                def _vt(ds):
                    return nc.sync.dma_start(
                        out=vt_st[:, ds, :], in_=vt[ds * 128 : (ds + 1) * 128, :]
                    ) if ds % 2 == 0 else nc.scalar.dma_start(
                        out=vt_st[:, ds, :], in_=vt[ds * 128 : (ds + 1) * 128, :]
                    )

                _vt(0)
                _vt(1)
                for ds in range(8):
                    nc.sync.dma_start(
                        out=wv_sb[:, ds, :], in_=wvt[ds * 128 : (ds + 1) * 128, :]
                    )
                for ds in range(2, 8):
                    _vt(ds)
                for ds in range(8):
                    nc.scalar.dma_start(
                        out=wq_sb[:, ds, :], in_=wqt[ds * 128 : (ds + 1) * 128, :]
                    )
                    nc.scalar.dma_start(
                        out=wk_sb[:, ds, :], in_=wkt[ds * 128 : (ds + 1) * 128, :]
                    )
                nc.scalar.dma_start(out=bq_sb, in_=bq)
                nc.scalar.dma_start(out=bk_sb, in_=bk)
                nc.sync.dma_start(out=bv_sb, in_=bv)
                nc.sync.dma_start(out=wo_sb, in_=wot)
                for ds in range(8):
                    nc.sync.dma_start(
                        out=kt_st[:, ds, :], in_=kt[ds * 128 : (ds + 1) * 128, :]
                    )
                    nc.scalar.dma_start(
                        out=qt_st[:, ds, :], in_=qt[ds * 128 : (ds + 1) * 128, :]
                    )

                dlyp = stage_ctx.enter_context(
                    tc.tile_pool(name="dlyp", bufs=2)
                )

                def emit_rest(root256, patches, hops=12):
                    """~3us PE-array rest as a true data dependency: a
                    serial DVE chain rooted at `root256` (an AP written
                    right where the rest should begin) produces exact
                    zeros, added in-place to `patches` ([P, 256] APs) --
                    the operand regions the next phase's first matmuls
                    read.  The list scheduler cannot hoist reads past
                    an in-place write to their operands."""
                    r = dlyp.tile([128, 256], BF16, name="dlyz")
                    nc.vector.tensor_copy(r, root256)
                    zt = dlyp.tile([128, 256], BF16, name="dlyz")
                    nc.vector.tensor_sub(zt, r, r)
                    for _ in range(hops):
                        t = dlyp.tile([128, 256], BF16, name="dlyz")
                        nc.vector.tensor_copy(t, zt)
                        zt = t
                    for p in patches:
                        pp_ = p.shape[0]
                        nc.vector.tensor_add(p, p, zt[0:pp_, :])

                # ---- Warmup: dummy matmuls while input DMAs land ----
                with tc.tile_pool(name="wps", bufs=1, space="PSUM") as wps:
                    wp = wps.tile([64, 128], F32, name="wp")
                    for _ in range(36):
                        nc.tensor.matmul(
                            wp, lhsT=wup[:, 0:64], rhs=wup,
                            start=True, stop=True,
                        )

                # ---- Phase A: v projection (s on partitions) ----
                bv3 = bv_sb.rearrange("p (h d) -> p h d", h=4)
                with tc.tile_pool(name="vps", bufs=4, space="PSUM") as vps:
                    for sg in range(4):  # groups of 4 s-chunks of 128
                        psv = [
                            vps.tile([128, LOC], F32, name="psv")
                            for _ in range(4)
                        ]
                        for ds in range(8):
                            for c in range(4):
                                s0 = sg * 512 + c * 128
                                nc.tensor.matmul(
                                    psv[c],
                                    lhsT=vt_st[:, ds, s0 : s0 + 128],
                                    rhs=wv_sb[:, ds, :],
                                    start=(ds == 0),
                                    stop=(ds == 7),
                                )
                        for c in range(4):
                            sc = sg * 4 + c
                            nc.vector.tensor_add(
                                v_sb[:, sc, :, 0:DH],
                                psv[c].rearrange("p (h d) -> p h d", h=4),
                                bv3,
                            )
                        if sg == 1:
                            emit_rest(
                                psv[3],
                                [vt_st[:, 0, 1024:1280],
                                 vt_st[:, 0, 1280:1536]],
                            )
                        if sg == 3:
                            emit_rest(
                                psv[3],
                                [qt_st[:, 0, 0:256], qt_st[:, 0, 256:512],
                                 kt_st[:, 0, 0:256], kt_st[:, 0, 256:512]],
                            )

                # ---- Phase B: q/k projections (local dims on partitions) ----
                with tc.tile_pool(name="qkps", bufs=4, space="PSUM") as qkps:
                    for st in range(4):  # s-tiles of 512
                        ps = {}
                        for t in range(2):
                            for pr in range(2):
                                ps[t, pr] = qkps.tile(
                                    [128, 512], F32, name="psqk"
                                )
                        for ds in range(8):
                            for pr in range(2):
                                nc.tensor.matmul(
                                    ps[0, pr],
                                    lhsT=wq_sb[:, ds,
                                               pr * 128 : (pr + 1) * 128],
                                    rhs=qt_st[:, ds,
                                              st * 512 : (st + 1) * 512],
                                    start=(ds == 0),
                                    stop=(ds == 7),
                                )
                                nc.tensor.matmul(
                                    ps[1, pr],
                                    lhsT=wk_sb[:, ds,
                                               pr * 128 : (pr + 1) * 128],
                                    rhs=kt_st[:, ds,
                                              st * 512 : (st + 1) * 512],
                                    start=(ds == 0),
                                    stop=(ds == 7),
                                )
                        # PSUM -> SBUF bf16 with per-partition bias, off ACT
                        for pr in range(2):
                            nc.vector.tensor_scalar_add(
                                qt_sb[:, pr, st * 512 : (st + 1) * 512],
                                ps[0, pr],
                                bq_sb[:, pr : pr + 1],
                            )
                            nc.scalar.activation(
                                out=kt_sb[:, pr, st * 512 : (st + 1) * 512],
                                in_=ps[1, pr],
                                func=AF.Identity,
                                bias=bk_sb[:, pr : pr + 1],
                                scale=1.0,
                            )
                        if st == 1:
                            emit_rest(
                                ps[1, 1][:, 0:256],
                                [qt_st[:, 0, 1024:1280],
                                 qt_st[:, 0, 1280:1536],
                                 kt_st[:, 0, 1024:1280],
                                 kt_st[:, 0, 1280:1536]],
                            )

            # ---- Phase C: attention (scores^T, exp, ones-augmented A@V) -----
            with (
                tc.tile_pool(name="expp", bufs=4) as expp,
                tc.tile_pool(name="qk2ps", bufs=2, space="PSUM") as qk2ps,
                tc.tile_pool(name="avps", bufs=2, space="PSUM") as avps,
                tc.tile_pool(name="nrm", bufs=4) as nrm,
                tc.tile_pool(name="dly", bufs=2) as dly,
            ):
                # ~3.5us PE-array rest before the attention phase so the HAM
                # clock gate re-promotes: a serial DVE chain rooted at B's
                # last output produces exact zeros, which are then added
                # in-place to the very qt_sb region C0h0's first matmuls
                # read.  The list scheduler cannot reorder around this (it
                # is a true data dependency into the matmul operands).
                zcur = dly.tile([128, 512], BF16, name="dlyt")
                nc.vector.tensor_sub(
                    zcur,
                    kt_sb[:, 1, 3 * 512 : 4 * 512],
                    kt_sb[:, 1, 3 * 512 : 4 * 512],
                )
                for _ in range(22):
                    t = dly.tile([128, 512], BF16, name="dlyt")
                    nc.vector.tensor_copy(t, zcur)
                    zcur = t
                nc.vector.tensor_add(
                    qt_sb[:, 0, 0:512], qt_sb[:, 0, 0:512], zcur
                )
                nc.vector.tensor_add(
                    qt_sb[:, 0, 512:1024], qt_sb[:, 0, 512:1024], zcur
                )
                for pr, ih in [(0, 0), (1, 0), (0, 1), (1, 1)]:
                    for hh in range(2):
                        h = 2 * pr + hh
                        r0, r1 = hh * 64, (hh + 1) * 64
                        psav = [
                            avps.tile([DH + 1, 512], F32, name="psav")
                            for _ in range(2)
                        ]

                        # one-deep software pipeline: AV(jc) is emitted after
                        # QK(jc+1) so the PE always has QK work in flight
                        # while ScalarE computes exp
                        def emit_qk(jc):
                            psqk = qk2ps.tile([128, 1024], F32, name="psqk2")
                            for it in range(2):
                                i0 = ih * 1024 + it * 512
                                nc.tensor.matmul(
                                    psqk[:, it * 512 : (it + 1) * 512],
                                    lhsT=kt_sb[r0:r1, pr,
                                               jc * 128 : (jc + 1) * 128],
                                    rhs=qt_sb[r0:r1, pr, i0 : i0 + 512],
                                    start=True,
                                    stop=True,
                                )
                            return psqk

                        def emit_exp_av(psqk, jc):
                            ex = expp.tile([128, 1024], BF16, name="ex")
                            nc.scalar.activation(
                                out=ex, in_=psqk, func=AF.Exp, scale=SCALE
                            )
                            for it in range(2):
                                nc.tensor.matmul(
                                    psav[it],
                                    lhsT=v_sb[:, jc, h, :],
                                    rhs=ex[:, it * 512 : (it + 1) * 512],
                                    start=(jc == 0),
                                    stop=(jc == 15),
                                )

                        prev = emit_qk(0)
                        for jc in range(1, 16):
                            cur = emit_qk(jc)
                            emit_exp_av(prev, jc - 1)
                            prev = cur
                        emit_exp_av(prev, 15)

                        # per-pass normalization, fully off ScalarE and
                        # overlapped with the next pass's matmuls
                        for it in range(2):
                            i0 = ih * 1024 + it * 512
                            rsum = nrm.tile([1, 512], F32, name="rsum")
                            nc.vector.tensor_copy(
                                rsum, psav[it][DH : DH + 1, :]
                            )
                            rrow = nrm.tile([1, 512], F32, name="rrow")
                            nc.vector.reciprocal_approx_fast(
                                out=rrow, in_=rsum
                            )
                            rb = nrm.tile([64, 512], F32, name="rb")
                            nc.gpsimd.partition_broadcast(rb, rrow)
                            nc.vector.tensor_mul(
                                ctx_sb[:, h, i0 : i0 + 512],
                                psav[it][0:DH, :],
                                rb,
                            )

            # ---- Phase D: output projection (partial over local dims) ------
            with (
                tc.tile_pool(name="pob", bufs=4) as pob,
                tc.tile_pool(name="pps", bufs=4, space="PSUM") as pps,
                tc.tile_pool(name="dly2", bufs=2) as dly2,
            ):
                # mid-D rest, same zero-patch trick: gate st2/st3's hc=0
                # operand region of ctx behind a DVE delay chain rooted at
                # the last st1 output tile
                def emit_rest_d(root, lo, hi):
                    zcur = dly2.tile([128, 512], BF16, name="dlyt2")
                    nc.vector.tensor_sub(zcur, root, root)
                    for _ in range(20):
                        t = dly2.tile([128, 512], BF16, name="dlyt2")
                        nc.vector.tensor_copy(t, zcur)
                        zcur = t
                    nc.vector.tensor_add(
                        ctx_sb[:, 0, lo:hi],
                        ctx_sb[:, 0, lo:hi],
                        zcur[0:64, :],
                    )

                last_ob = None
                for st in range(4):  # s-tiles of 512
                    for ec in range(8):  # output-dim chunks of 128
                        pp = pps.tile([128, 512], F32, name="pp")
                        for hc in range(4):
                            nc.tensor.matmul(
                                pp,
                                lhsT=wo_sb[:, hc, ec * 128 : (ec + 1) * 128],
                                rhs=ctx_sb[:, hc, st * 512 : (st + 1) * 512],
                                start=(hc == 0),
                                stop=(hc == 3),
                            )
                        ob = pob.tile([128, 512], F32, name="ob")
                        if ec % 2 == 0:
                            nc.vector.tensor_copy(ob, pp)
                            nc.sync.dma_start(
                                out=outp[ec * 128 : (ec + 1) * 128,
                                         st * 512 : (st + 1) * 512],
                                in_=ob,
                            )
                        else:
                            nc.scalar.activation(out=ob, in_=pp, func=AF.Copy)
                            nc.scalar.dma_start(
                                out=outp[ec * 128 : (ec + 1) * 128,
                                         st * 512 : (st + 1) * 512],
                                in_=ob,
                            )
                        last_ob = ob
                    if st == 1:
                        emit_rest_d(last_ob, 1024, 1536)
                    elif st == 2:
                        emit_rest_d(last_ob, 1536, 2048)

    nc.compile()
    return nc


def _get_nc():
    global _CACHED_NC
    if _CACHED_NC is None:
        _CACHED_NC = build_nc()
    return _CACHED_NC


def make_in_maps(Q, K, V, W_q, b_q, W_k, b_k, W_v, b_v, W_o):
    import ml_dtypes

    BF = ml_dtypes.bfloat16
    xt = {}
    for b in range(B):
        xt["q", b] = np.ascontiguousarray(np.asarray(Q[b], np.float32).T).astype(BF)
        xt["k", b] = np.ascontiguousarray(np.asarray(K[b], np.float32).T).astype(BF)
        xt["v", b] = np.ascontiguousarray(np.asarray(V[b], np.float32).T).astype(BF)
    in_maps = []
    for c in range(NCORES):
        b, g = divmod(c, 4)
        L = slice(g * LOC, (g + 1) * LOC)
        wqt = np.ascontiguousarray(np.asarray(W_q, np.float32)[L, :].T).astype(BF)
        wkt = np.ascontiguousarray(np.asarray(W_k, np.float32)[L, :].T).astype(BF)
        wvt = np.ascontiguousarray(np.asarray(W_v, np.float32)[L, :].T).astype(BF)
        wot = np.ascontiguousarray(
            np.asarray(W_o, np.float32)[:, L].T.reshape(4, DH, D)
            .transpose(1, 0, 2).astype(BF)
        )
        bqh = np.ascontiguousarray(np.asarray(b_q, np.float32)[L].reshape(2, 128).T)
        bkh = np.ascontiguousarray(np.asarray(b_k, np.float32)[L].reshape(2, 128).T)
        bvh = np.ascontiguousarray(
            np.broadcast_to(np.asarray(b_v, np.float32)[L], (128, LOC))
        )
        in_maps.append(
            dict(
                qt=xt["q", b], kt=xt["k", b], vt=xt["v", b],
                wqt=wqt, wkt=wkt, wvt=wvt, wot=wot,
                bq=bqh, bk=bkh, bv=bvh,
            )
        )
    return in_maps


def gather(results, b_o):
    out = np.zeros((B, S, D), dtype=np.float32)
    for c in range(NCORES):
        b = c // 4
        out[b] += results[c]["outp"].T
    out += np.asarray(b_o, np.float32)
    return out


def kernel(Q, K, V, W_q, b_q, W_k, b_k, W_v, b_v, W_o, b_o):
    nc = _get_nc()
    in_maps = make_in_maps(Q, K, V, W_q, b_q, W_k, b_k, W_v, b_v, W_o)
    res = run_bass_kernel_spmd(nc, in_maps, core_ids=list(range(NCORES)))
    return gather(res.results, b_o)


# revision 22
# speedup vs baseline: 1.1844x; 1.1706x over previous
"""Multi-head attention (B=2, S=2048, D=1024, H=16) on 8 Trainium2 cores.

Sharding: core c handles (batch b = c//4, head-group g = c%4 of 4 heads).
Megatron-style: W_q/k/v rows (output dims) column-sharded per head-group;
W_o columns row-sharded; the all-reduce over head-groups happens on the host
at gather time (sum of 4 partial projections per batch), where b_o is added.

All device data is bf16 (fp32 PSUM accumulation).  Inputs X^T are staged
whole-row ([128, 2048] = 256 KB contiguous DMAs) into SBUF ahead of the
compute that reads them, so no phase is DMA-bound.

The PE HAM activity governor on this part throttles the PE clock to 1.2 GHz
(K=4/8) under sustained near-100% PE duty, granting at most ~17us of 2.4 GHz
at a time; re-promotion needs idle in the window mix, and long saturated
stretches can lock the gate cold for 50-300us.  The kernel therefore keeps a
burst/idle rhythm the gate tolerates: in phase C, avps bufs=2 makes each
pass's first A@V wait on the previous pass's normalization chain (~2-4us of
genuine array idle per ~20us pass, nearly free because the ScalarE exp
stream is the phase bottleneck), and the B->C boundary has an explicit rest:
a serial DVE chain rooted at B's last output computes exact zeros that are
added in-place to the qt_sb region C's first matmuls read (a true data
dependency the list scheduler cannot hoist reads around).

Device layout (per core):
  Phase A: v projection (seq on partitions), v stored [128, 16 s-chunks,
  4 heads, 65] with a ones column per head so A@V accumulates softmax
  row-sums in PSUM row 64.
  Phase B: q/k projections transposed: q^T,k^T [256, 2048] as [128, 2, S].
  Phase C: per (pr, ih, hh) pass: scores^T [j, i] via K=64 matmuls (row
  pairs via base partitions 0/64), exp on ScalarE straight out of PSUM
  (scale=1/8, no max subtraction: scores ~ N(0,1)), ones-augmented A@V.
  ScalarE does nothing but exp; per-pass normalization: DVE rowsum copy ->
  DVE reciprocal_approx_fast -> gpsimd partition broadcast -> fused DVE
  (PSUM * rinv) -> bf16 ctx write (hh=1 lands via a partition-shifting
  SBUF->SBUF DMA so both heads of a pair stack on 128 partitions).
  Phase D: output projection with head-pair-stacked ctx [128, 2, S] and
  W_o^T [128, 2, 1024]: K=128 matmuls, half the instructions of a K=64
  layout.  st0/st1 tiles are emitted right after their inputs complete and
  overlap the last two attention blocks; only st2/st3 (~7us) trail.
"""

import numpy as np
from contextlib import ExitStack

import concourse.bass as bass
import concourse.bacc as bacc
import concourse.tile as tile
from concourse import mybir
from concourse.bass_utils import run_bass_kernel_spmd

F32 = mybir.dt.float32
BF16 = mybir.dt.bfloat16
AF = mybir.ActivationFunctionType

B, S, D = 2, 2048, 1024
H, DH = 16, 64
NCORES = 8
LOC = D // 4          # 256 local dims per head-group
SCALE = 1.0 / np.sqrt(DH)

_CACHED_NC = None


def build_nc():
    nc = bacc.Bacc("TRN2", target_bir_lowering=False, debug=False)

    qt = nc.dram_tensor("qt", [D, S], BF16, kind="ExternalInput").ap()
    kt = nc.dram_tensor("kt", [D, S], BF16, kind="ExternalInput").ap()
    vt = nc.dram_tensor("vt", [D, S], BF16, kind="ExternalInput").ap()
    wqt = nc.dram_tensor("wqt", [D, LOC], BF16, kind="ExternalInput").ap()
    wkt = nc.dram_tensor("wkt", [D, LOC], BF16, kind="ExternalInput").ap()
    wvt = nc.dram_tensor("wvt", [D, LOC], BF16, kind="ExternalInput").ap()
    wot = nc.dram_tensor("wot", [128, 2, D], BF16, kind="ExternalInput").ap()
    bq = nc.dram_tensor("bq", [128, 2], F32, kind="ExternalInput").ap()
    bk = nc.dram_tensor("bk", [128, 2], F32, kind="ExternalInput").ap()
    bv = nc.dram_tensor("bv", [128, LOC], F32, kind="ExternalInput").ap()
    outp = nc.dram_tensor("outp", [D, S], F32, kind="ExternalOutput").ap()

    with tile.TileContext(nc) as tc:
        with ExitStack() as ctx:
            wsb = ctx.enter_context(tc.tile_pool(name="wsb", bufs=1))
            big = ctx.enter_context(tc.tile_pool(name="big", bufs=1))

            # persistent SBUF state
            qt_sb = big.tile([128, 2, S], BF16, name="qt_sb")
            kt_sb = big.tile([128, 2, S], BF16, name="kt_sb")
            v_sb = big.tile([128, 16, 4, DH + 1], BF16, name="v_sb")
            # head-pair-stacked context: [0:64] = head 2*pr, [64:128] =
            # head 2*pr+1, so phase D contracts K=128 per head-pair
            ctx2 = big.tile([128, 2, S], BF16, name="ctx2")

            wq_sb = wsb.tile([128, 8, LOC], BF16, name="wq_sb")
            wk_sb = wsb.tile([128, 8, LOC], BF16, name="wk_sb")
            wv_sb = wsb.tile([128, 8, LOC], BF16, name="wv_sb")
            wo_sb = wsb.tile([128, 2, D], BF16, name="wo_sb")
            bq_sb = wsb.tile([128, 2], F32, name="bq_sb")
            bk_sb = wsb.tile([128, 2], F32, name="bk_sb")
            bv_sb = wsb.tile([128, LOC], F32, name="bv_sb")
            wup = wsb.tile([64, 128], BF16, name="wup")

            nc.gpsimd.memset(wup, 0.0)
            # ones column of v (accumulates softmax row-sums in A@V)
            nc.gpsimd.memset(v_sb[:, :, :, DH : DH + 1], 1.0)

            with ExitStack() as stage_ctx:
                stage = stage_ctx.enter_context(
                    tc.tile_pool(name="stage", bufs=1)
                )
                # whole-row input staging, both HW queues (~160 GB/s each):
                # vt first (phase A), then q/k weights, then qt/kt ahead of
                # phase B's reads
                vt_st = stage.tile([128, 8, S], BF16, name="vt_st")
                qt_st = stage.tile([128, 8, S], BF16, name="qt_st")
                kt_st = stage.tile([128, 8, S], BF16, name="kt_st")

                def _vt(ds):
                    eng = nc.sync if ds % 2 == 0 else nc.scalar
                    eng.dma_start(
                        out=vt_st[:, ds, :],
                        in_=vt[ds * 128 : (ds + 1) * 128, :],
                    )

                _vt(0)
                _vt(1)
                for ds in range(8):
                    nc.sync.dma_start(
                        out=wv_sb[:, ds, :], in_=wvt[ds * 128 : (ds + 1) * 128, :]
                    )
                for ds in range(2, 8):
                    _vt(ds)
                for ds in range(8):
                    nc.scalar.dma_start(
                        out=wq_sb[:, ds, :], in_=wqt[ds * 128 : (ds + 1) * 128, :]
                    )
                    nc.scalar.dma_start(
                        out=wk_sb[:, ds, :], in_=wkt[ds * 128 : (ds + 1) * 128, :]
                    )
                nc.scalar.dma_start(out=bq_sb, in_=bq)
                nc.scalar.dma_start(out=bk_sb, in_=bk)
                nc.sync.dma_start(out=bv_sb, in_=bv)
                nc.sync.dma_start(out=wo_sb, in_=wot)
                for ds in range(8):
                    nc.sync.dma_start(
                        out=kt_st[:, ds, :], in_=kt[ds * 128 : (ds + 1) * 128, :]
                    )
                    nc.scalar.dma_start(
                        out=qt_st[:, ds, :], in_=qt[ds * 128 : (ds + 1) * 128, :]
                    )

                # ---- Warmup: dummy matmuls while input DMAs land ----
                with tc.tile_pool(name="wps", bufs=1, space="PSUM") as wps:
                    wp = wps.tile([64, 128], F32, name="wp")
                    for _ in range(36):
                        nc.tensor.matmul(
                            wp, lhsT=wup[:, 0:64], rhs=wup,
                            start=True, stop=True,
                        )

                # ---- Phase A: v projection (s on partitions) ----
                bv3 = bv_sb.rearrange("p (h d) -> p h d", h=4)
                with tc.tile_pool(name="vps", bufs=4, space="PSUM") as vps:
                    for sg in range(4):  # groups of 4 s-chunks of 128
                        psv = [
                            vps.tile([128, LOC], F32, name="psv")
                            for _ in range(4)
                        ]
                        for ds in range(8):
                            for c in range(4):
                                s0 = sg * 512 + c * 128
                                nc.tensor.matmul(
                                    psv[c],
                                    lhsT=vt_st[:, ds, s0 : s0 + 128],
                                    rhs=wv_sb[:, ds, :],
                                    start=(ds == 0),
                                    stop=(ds == 7),
                                )
                        for c in range(4):
                            sc = sg * 4 + c
                            nc.vector.tensor_add(
                                v_sb[:, sc, :, 0:DH],
                                psv[c].rearrange("p (h d) -> p h d", h=4),
                                bv3,
                            )

                # ---- Phase B: q/k projections (local dims on partitions) ----
                with tc.tile_pool(name="qkps", bufs=4, space="PSUM") as qkps:
                    for st in range(4):  # s-tiles of 512
                        ps = {}
                        for t in range(2):
                            for pr in range(2):
                                ps[t, pr] = qkps.tile(
                                    [128, 512], F32, name="psqk"
                                )
                        for ds in range(8):
                            for pr in range(2):
                                nc.tensor.matmul(
                                    ps[0, pr],
                                    lhsT=wq_sb[:, ds,
                                               pr * 128 : (pr + 1) * 128],
                                    rhs=qt_st[:, ds,
                                              st * 512 : (st + 1) * 512],
                                    start=(ds == 0),
                                    stop=(ds == 7),
                                )
                                nc.tensor.matmul(
                                    ps[1, pr],
                                    lhsT=wk_sb[:, ds,
                                               pr * 128 : (pr + 1) * 128],
                                    rhs=kt_st[:, ds,
                                              st * 512 : (st + 1) * 512],
                                    start=(ds == 0),
                                    stop=(ds == 7),
                                )
                        # PSUM -> SBUF bf16 with per-partition bias, off ACT
                        for pr in range(2):
                            nc.vector.tensor_scalar_add(
                                qt_sb[:, pr, st * 512 : (st + 1) * 512],
                                ps[0, pr],
                                bq_sb[:, pr : pr + 1],
                            )
                            nc.vector.tensor_scalar_add(
                                kt_sb[:, pr, st * 512 : (st + 1) * 512],
                                ps[1, pr],
                                bk_sb[:, pr : pr + 1],
                            )

            # ---- Phase C: attention + interleaved phase-D head ----
            with (
                tc.tile_pool(name="expp", bufs=4) as expp,
                tc.tile_pool(name="qk2ps", bufs=2, space="PSUM") as qk2ps,
                tc.tile_pool(name="avps", bufs=2, space="PSUM") as avps,
                tc.tile_pool(name="nrm", bufs=4) as nrm,
                tc.tile_pool(name="dly", bufs=2) as dly,
                tc.tile_pool(name="ctmp", bufs=4) as ctmp,
                tc.tile_pool(name="pps", bufs=2, space="PSUM") as pps,
                tc.tile_pool(name="pob", bufs=4) as pob,
            ):
                # B->C rest: ~4us PE-array idle as a true data dependency
                # (serial DVE chain -> exact zeros added in-place to the
                # qt_sb region C's first matmuls read)
                zcur = dly.tile([128, 512], BF16, name="dlyt")
                nc.vector.tensor_sub(
                    zcur,
                    kt_sb[:, 1, 3 * 512 : 4 * 512],
                    kt_sb[:, 1, 3 * 512 : 4 * 512],
                )
                for _ in range(10):
                    t = dly.tile([128, 512], BF16, name="dlyt")
                    nc.vector.tensor_copy(t, zcur)
                    zcur = t
                nc.vector.tensor_add(
                    qt_sb[:, 0, 0:512], qt_sb[:, 0, 0:512], zcur
                )
                nc.vector.tensor_add(
                    qt_sb[:, 0, 512:1024], qt_sb[:, 0, 512:1024], zcur
                )

                def emit_d(st, engs):
                    """One phase-D s-tile: 8 output chunks x K=128 over the
                    two head-pairs."""
                    for ec in range(8):
                        pp = pps.tile([128, 512], F32, name="pp")
                        for hp in range(2):
                            nc.tensor.matmul(
                                pp,
                                lhsT=wo_sb[:, hp, ec * 128 : (ec + 1) * 128],
                                rhs=ctx2[:, hp, st * 512 : (st + 1) * 512],
                                start=(hp == 0),
                                stop=(hp == 1),
                            )
                        ob = pob.tile([128, 512], F32, name="ob")
                        engs[ec % len(engs)][0](ob, pp)
                        engs[ec % len(engs)][1](
                            out=outp[ec * 128 : (ec + 1) * 128,
                                     st * 512 : (st + 1) * 512],
                            in_=ob,
                        )

                dve_copy = lambda o, i: nc.vector.tensor_copy(o, i)
                act_copy = lambda o, i: nc.scalar.activation(
                    out=o, in_=i, func=AF.Copy
                )

                for blk, (pr, ih) in enumerate(
                    [(0, 0), (1, 0), (0, 1), (1, 1)]
                ):
                    for hh in range(2):
                        h = 2 * pr + hh
                        r0, r1 = hh * 64, (hh + 1) * 64
                        psav = [
                            avps.tile([DH + 1, 512], F32, name="psav")
                            for _ in range(2)
                        ]

                        # one-deep software pipeline: AV(jc) is emitted after
                        # QK(jc+1) so the PE always has QK work in flight
                        # while ScalarE computes exp
                        def emit_qk(jc):
                            psqk = qk2ps.tile([128, 1024], F32, name="psqk2")
                            for it in range(2):
                                i0 = ih * 1024 + it * 512
                                nc.tensor.matmul(
                                    psqk[:, it * 512 : (it + 1) * 512],
                                    lhsT=kt_sb[r0:r1, pr,
                                               jc * 128 : (jc + 1) * 128],
                                    rhs=qt_sb[r0:r1, pr, i0 : i0 + 512],
                                    start=True,
                                    stop=True,
                                )
                            return psqk

                        def emit_exp_av(psqk, jc):
                            ex = expp.tile([128, 1024], BF16, name="ex")
                            nc.scalar.activation(
                                out=ex, in_=psqk, func=AF.Exp, scale=SCALE
                            )
                            for it in range(2):
                                nc.tensor.matmul(
                                    psav[it],
                                    lhsT=v_sb[:, jc, h, :],
                                    rhs=ex[:, it * 512 : (it + 1) * 512],
                                    start=(jc == 0),
                                    stop=(jc == 15),
                                )

                        prev = emit_qk(0)
                        for jc in range(1, 16):
                            cur = emit_qk(jc)
                            emit_exp_av(prev, jc - 1)
                            prev = cur
                        emit_exp_av(prev, 15)

                        # per-pass normalization, fully off ScalarE and
                        # overlapped; its chain also paces the next pass's
                        # psav reuse, giving the HAM gate its idle window
                        for it in range(2):
                            i0 = ih * 1024 + it * 512
                            rsum = nrm.tile([1, 512], F32, name="rsum")
                            nc.vector.tensor_copy(
                                rsum, psav[it][DH : DH + 1, :]
                            )
                            rrow = nrm.tile([1, 512], F32, name="rrow")
                            nc.vector.reciprocal_approx_fast(
                                out=rrow, in_=rsum
                            )
                            rb = nrm.tile([64, 512], F32, name="rb")
                            nc.gpsimd.partition_broadcast(rb, rrow)
                            if hh == 0:
                                nc.vector.tensor_mul(
                                    ctx2[0:64, pr, i0 : i0 + 512],
                                    psav[it][0:DH, :],
                                    rb,
                                )
                            else:
                                # DVE lanes cannot shift partitions; land in
                                # a tmp and let a SBUF->SBUF DMA place head
                                # 2*pr+1 on partitions 64..127
                                ct = ctmp.tile([64, 512], BF16, name="ct")
                                nc.vector.tensor_mul(
                                    ct, psav[it][0:DH, :], rb
                                )
                                nc.sync.dma_start(
                                    out=ctx2[64:128, pr, i0 : i0 + 512],
                                    in_=ct,
                                )

                    # phase-D head: st0/st1 become ready once both ih=0
                    # blocks (blk 0 and 1) are normalized
                    if blk == 1:
                        emit_d(0, [(dve_copy, nc.sync.dma_start)])
                    elif blk == 2:
                        emit_d(1, [(dve_copy, nc.sync.dma_start)])

            # ---- Phase D tail: st2/st3 ----
            with (
                tc.tile_pool(name="pob2", bufs=4) as pob2,
                tc.tile_pool(name="pps2", bufs=4, space="PSUM") as pps2,
            ):
                for st in (2, 3):
                    for ec in range(8):
                        pp = pps2.tile([128, 512], F32, name="pp2")
                        for hp in range(2):
                            nc.tensor.matmul(
                                pp,
                                lhsT=wo_sb[:, hp, ec * 128 : (ec + 1) * 128],
                                rhs=ctx2[:, hp, st * 512 : (st + 1) * 512],
                                start=(hp == 0),
                                stop=(hp == 1),
                            )
                        ob = pob2.tile([128, 512], F32, name="ob2")
                        if ec % 2 == 0:
                            nc.vector.tensor_copy(ob, pp)
                            nc.sync.dma_start(
                                out=outp[ec * 128 : (ec + 1) * 128,
                                         st * 512 : (st + 1) * 512],
                                in_=ob,
                            )
                        else:
                            nc.scalar.activation(out=ob, in_=pp, func=AF.Copy)
                            nc.scalar.dma_start(
                                out=outp[ec * 128 : (ec + 1) * 128,
                                         st * 512 : (st + 1) * 512],
                                in_=ob,
                            )

    nc.compile()
    return nc


def _get_nc():
    global _CACHED_NC
    if _CACHED_NC is None:
        _CACHED_NC = build_nc()
    return _CACHED_NC


def make_in_maps(Q, K, V, W_q, b_q, W_k, b_k, W_v, b_v, W_o):
    import ml_dtypes

    BF = ml_dtypes.bfloat16
    xt = {}
    for b in range(B):
        xt["q", b] = np.ascontiguousarray(np.asarray(Q[b], np.float32).T).astype(BF)
        xt["k", b] = np.ascontiguousarray(np.asarray(K[b], np.float32).T).astype(BF)
        xt["v", b] = np.ascontiguousarray(np.asarray(V[b], np.float32).T).astype(BF)
    in_maps = []
    for c in range(NCORES):
        b, g = divmod(c, 4)
        L = slice(g * LOC, (g + 1) * LOC)
        wqt = np.ascontiguousarray(np.asarray(W_q, np.float32)[L, :].T).astype(BF)
        wkt = np.ascontiguousarray(np.asarray(W_k, np.float32)[L, :].T).astype(BF)
        wvt = np.ascontiguousarray(np.asarray(W_v, np.float32)[L, :].T).astype(BF)
        # head-pair-stacked W_o^T: wot[p, hp, e] = W_o[e, g*256 + hp*128 + p]
        wot = np.ascontiguousarray(
            np.asarray(W_o, np.float32)[:, L].T.reshape(2, 128, D)
            .transpose(1, 0, 2).astype(BF)
        )
        bqh = np.ascontiguousarray(np.asarray(b_q, np.float32)[L].reshape(2, 128).T)
        bkh = np.ascontiguousarray(np.asarray(b_k, np.float32)[L].reshape(2, 128).T)
        bvh = np.ascontiguousarray(
            np.broadcast_to(np.asarray(b_v, np.float32)[L], (128, LOC))
        )
        in_maps.append(
            dict(
                qt=xt["q", b], kt=xt["k", b], vt=xt["v", b],
                wqt=wqt, wkt=wkt, wvt=wvt, wot=wot,
                bq=bqh, bk=bkh, bv=bvh,
            )
        )
    return in_maps


def gather(results, b_o):
    out = np.zeros((B, S, D), dtype=np.float32)
    for c in range(NCORES):
        b = c // 4
        out[b] += results[c]["outp"].T
    out += np.asarray(b_o, np.float32)
    return out


def kernel(Q, K, V, W_q, b_q, W_k, b_k, W_v, b_v, W_o, b_o):
    nc = _get_nc()
    in_maps = make_in_maps(Q, K, V, W_q, b_q, W_k, b_k, W_v, b_v, W_o)
    res = run_bass_kernel_spmd(nc, in_maps, core_ids=list(range(NCORES)))
    return gather(res.results, b_o)


# revision 23
# speedup vs baseline: 1.1882x; 1.0032x over previous
"""Multi-head attention (B=2, S=2048, D=1024, H=16) on 8 Trainium2 cores.

Sharding: core c handles (batch b = c//4, head-group g = c%4 of 4 heads).
Megatron-style: W_q/k/v rows (output dims) column-sharded per head-group;
W_o columns row-sharded; the all-reduce over head-groups happens on the host
at gather time (sum of 4 partial projections per batch), where b_o is added.

All device data is bf16 (fp32 PSUM accumulation).  Inputs X^T are staged
whole-row ([128, 2048] = 256 KB contiguous DMAs) into SBUF ahead of the
compute that reads them, so no phase is DMA-bound.

The PE HAM activity governor on this part throttles the PE clock to 1.2 GHz
(K=4/8) under sustained near-100% PE duty, granting at most ~17us of 2.4 GHz
at a time; re-promotion needs idle in the window mix, and long saturated
stretches can lock the gate cold for 50-300us.  The kernel therefore keeps a
burst/idle rhythm the gate tolerates: in phase C, avps bufs=2 makes each
pass's first A@V wait on the previous pass's normalization chain (~2-4us of
genuine array idle per ~20us pass, nearly free because the ScalarE exp
stream is the phase bottleneck), and the B->C boundary has an explicit rest:
a serial DVE chain rooted at B's last output computes exact zeros that are
added in-place to the qt_sb region C's first matmuls read (a true data
dependency the list scheduler cannot hoist reads around).

Device layout (per core):
  Phase A: v projection (seq on partitions), v stored [128, 16 s-chunks,
  4 heads, 65] with a ones column per head so A@V accumulates softmax
  row-sums in PSUM row 64.
  Phase B: q/k projections transposed: q^T,k^T [256, 2048] as [128, 2, S].
  Phase C: per (pr, ih, hh) pass: scores^T [j, i] via K=64 matmuls (row
  pairs via base partitions 0/64), exp on ScalarE straight out of PSUM
  (scale=1/8, no max subtraction: scores ~ N(0,1)), ones-augmented A@V.
  ScalarE does nothing but exp; per-pass normalization: DVE rowsum copy ->
  DVE reciprocal_approx_fast -> gpsimd partition broadcast -> fused DVE
  (PSUM * rinv) -> bf16 ctx write (hh=1 lands via a partition-shifting
  SBUF->SBUF DMA so both heads of a pair stack on 128 partitions).
  Phase D: output projection with head-pair-stacked ctx [128, 2, S] and
  W_o^T [128, 2, 1024]: K=128 matmuls, half the instructions of a K=64
  layout.  st0/st1 tiles are emitted right after their inputs complete and
  overlap the last two attention blocks; only st2/st3 (~7us) trail.
"""

import numpy as np
from contextlib import ExitStack

import concourse.bass as bass
import concourse.bacc as bacc
import concourse.tile as tile
from concourse import mybir
from concourse.bass_utils import run_bass_kernel_spmd

F32 = mybir.dt.float32
BF16 = mybir.dt.bfloat16
AF = mybir.ActivationFunctionType

B, S, D = 2, 2048, 1024
H, DH = 16, 64
NCORES = 8
LOC = D // 4          # 256 local dims per head-group
SCALE = 1.0 / np.sqrt(DH)

_CACHED_NC = None


def build_nc():
    nc = bacc.Bacc("TRN2", target_bir_lowering=False, debug=False)

    qt = nc.dram_tensor("qt", [D, S], BF16, kind="ExternalInput").ap()
    kt = nc.dram_tensor("kt", [D, S], BF16, kind="ExternalInput").ap()
    vt = nc.dram_tensor("vt", [D, S], BF16, kind="ExternalInput").ap()
    wqt = nc.dram_tensor("wqt", [D, LOC], BF16, kind="ExternalInput").ap()
    wkt = nc.dram_tensor("wkt", [D, LOC], BF16, kind="ExternalInput").ap()
    wvt = nc.dram_tensor("wvt", [D, LOC], BF16, kind="ExternalInput").ap()
    wot = nc.dram_tensor("wot", [128, 2, D], BF16, kind="ExternalInput").ap()
    bq = nc.dram_tensor("bq", [128, 2], F32, kind="ExternalInput").ap()
    bk = nc.dram_tensor("bk", [128, 2], F32, kind="ExternalInput").ap()
    bv = nc.dram_tensor("bv", [128, LOC], F32, kind="ExternalInput").ap()
    outp = nc.dram_tensor("outp", [D, S], F32, kind="ExternalOutput").ap()

    with tile.TileContext(nc) as tc:
        with ExitStack() as ctx:
            wsb = ctx.enter_context(tc.tile_pool(name="wsb", bufs=1))
            big = ctx.enter_context(tc.tile_pool(name="big", bufs=1))

            # persistent SBUF state
            qt_sb = big.tile([128, 2, S], BF16, name="qt_sb")
            kt_sb = big.tile([128, 2, S], BF16, name="kt_sb")
            v_sb = big.tile([128, 16, 4, DH + 1], BF16, name="v_sb")
            # head-pair-stacked context: [0:64] = head 2*pr, [64:128] =
            # head 2*pr+1, so phase D contracts K=128 per head-pair
            ctx2 = big.tile([128, 2, S], BF16, name="ctx2")

            wq_sb = wsb.tile([128, 8, LOC], BF16, name="wq_sb")
            wk_sb = wsb.tile([128, 8, LOC], BF16, name="wk_sb")
            wv_sb = wsb.tile([128, 8, LOC], BF16, name="wv_sb")
            wo_sb = wsb.tile([128, 2, D], BF16, name="wo_sb")
            bq_sb = wsb.tile([128, 2], F32, name="bq_sb")
            bk_sb = wsb.tile([128, 2], F32, name="bk_sb")
            bv_sb = wsb.tile([128, LOC], F32, name="bv_sb")
            wup = wsb.tile([64, 128], BF16, name="wup")

            nc.gpsimd.memset(wup, 0.0)
            # ones column of v (accumulates softmax row-sums in A@V)
            nc.gpsimd.memset(v_sb[:, :, :, DH : DH + 1], 1.0)

            with ExitStack() as stage_ctx:
                stage = stage_ctx.enter_context(
                    tc.tile_pool(name="stage", bufs=1)
                )
                # whole-row input staging, both HW queues (~160 GB/s each):
                # vt first (phase A), then q/k weights, then qt/kt ahead of
                # phase B's reads
                vt_st = stage.tile([128, 8, S], BF16, name="vt_st")
                qt_st = stage.tile([128, 8, S], BF16, name="qt_st")
                kt_st = stage.tile([128, 8, S], BF16, name="kt_st")

                def _vt(ds):
                    eng = nc.sync if ds % 2 == 0 else nc.scalar
                    eng.dma_start(
                        out=vt_st[:, ds, :],
                        in_=vt[ds * 128 : (ds + 1) * 128, :],
                    )

                _vt(0)
                _vt(1)
                for ds in range(8):
                    nc.sync.dma_start(
                        out=wv_sb[:, ds, :], in_=wvt[ds * 128 : (ds + 1) * 128, :]
                    )
                for ds in range(2, 8):
                    _vt(ds)
                for ds in range(8):
                    nc.scalar.dma_start(
                        out=wq_sb[:, ds, :], in_=wqt[ds * 128 : (ds + 1) * 128, :]
                    )
                    nc.scalar.dma_start(
                        out=wk_sb[:, ds, :], in_=wkt[ds * 128 : (ds + 1) * 128, :]
                    )
                nc.scalar.dma_start(out=bq_sb, in_=bq)
                nc.scalar.dma_start(out=bk_sb, in_=bk)
                nc.sync.dma_start(out=bv_sb, in_=bv)
                nc.sync.dma_start(out=wo_sb, in_=wot)
                for ds in range(8):
                    nc.sync.dma_start(
                        out=kt_st[:, ds, :], in_=kt[ds * 128 : (ds + 1) * 128, :]
                    )
                    nc.scalar.dma_start(
                        out=qt_st[:, ds, :], in_=qt[ds * 128 : (ds + 1) * 128, :]
                    )

                # ---- Warmup: dummy matmuls while input DMAs land ----
                with tc.tile_pool(name="wps", bufs=1, space="PSUM") as wps:
                    wp = wps.tile([64, 128], F32, name="wp")
                    for _ in range(36):
                        nc.tensor.matmul(
                            wp, lhsT=wup[:, 0:64], rhs=wup,
                            start=True, stop=True,
                        )

                # ---- Phase A: v projection (s on partitions) ----
                bv3 = bv_sb.rearrange("p (h d) -> p h d", h=4)
                with tc.tile_pool(name="vps", bufs=8, space="PSUM") as vps:
                    for sg in range(4):  # groups of 4 s-chunks of 128
                        psv = [
                            vps.tile([128, LOC], F32, name="psv")
                            for _ in range(4)
                        ]
                        for ds in range(8):
                            for c in range(4):
                                s0 = sg * 512 + c * 128
                                nc.tensor.matmul(
                                    psv[c],
                                    lhsT=vt_st[:, ds, s0 : s0 + 128],
                                    rhs=wv_sb[:, ds, :],
                                    start=(ds == 0),
                                    stop=(ds == 7),
                                )
                        for c in range(4):
                            sc = sg * 4 + c
                            nc.vector.tensor_add(
                                v_sb[:, sc, :, 0:DH],
                                psv[c].rearrange("p (h d) -> p h d", h=4),
                                bv3,
                            )

                # ---- Phase B: q/k projections (local dims on partitions) ----
                with tc.tile_pool(name="qkps", bufs=8, space="PSUM") as qkps:
                    for st in range(4):  # s-tiles of 512
                        ps = {}
                        for t in range(2):
                            for pr in range(2):
                                ps[t, pr] = qkps.tile(
                                    [128, 512], F32, name="psqk"
                                )
                        for ds in range(8):
                            for pr in range(2):
                                nc.tensor.matmul(
                                    ps[0, pr],
                                    lhsT=wq_sb[:, ds,
                                               pr * 128 : (pr + 1) * 128],
                                    rhs=qt_st[:, ds,
                                              st * 512 : (st + 1) * 512],
                                    start=(ds == 0),
                                    stop=(ds == 7),
                                )
                                nc.tensor.matmul(
                                    ps[1, pr],
                                    lhsT=wk_sb[:, ds,
                                               pr * 128 : (pr + 1) * 128],
                                    rhs=kt_st[:, ds,
                                              st * 512 : (st + 1) * 512],
                                    start=(ds == 0),
                                    stop=(ds == 7),
                                )
                        # PSUM -> SBUF bf16 with per-partition bias, off ACT
                        for pr in range(2):
                            nc.vector.tensor_scalar_add(
                                qt_sb[:, pr, st * 512 : (st + 1) * 512],
                                ps[0, pr],
                                bq_sb[:, pr : pr + 1],
                            )
                            nc.scalar.activation(
                                out=kt_sb[:, pr, st * 512 : (st + 1) * 512],
                                in_=ps[1, pr],
                                func=AF.Identity,
                                bias=bk_sb[:, pr : pr + 1],
                                scale=1.0,
                            )

            # ---- Phase C: attention + interleaved phase-D head ----
            with (
                tc.tile_pool(name="expp", bufs=4) as expp,
                tc.tile_pool(name="qk2ps", bufs=2, space="PSUM") as qk2ps,
                tc.tile_pool(name="avps", bufs=2, space="PSUM") as avps,
                tc.tile_pool(name="nrm", bufs=4) as nrm,
                tc.tile_pool(name="dly", bufs=2) as dly,
                tc.tile_pool(name="ctmp", bufs=4) as ctmp,
                tc.tile_pool(name="pps", bufs=2, space="PSUM") as pps,
                tc.tile_pool(name="pob", bufs=4) as pob,
            ):
                # B->C rest: ~4us PE-array idle as a true data dependency
                # (serial DVE chain -> exact zeros added in-place to the
                # qt_sb region C's first matmuls read)
                zcur = dly.tile([128, 512], BF16, name="dlyt")
                nc.vector.tensor_sub(
                    zcur,
                    kt_sb[:, 1, 3 * 512 : 4 * 512],
                    kt_sb[:, 1, 3 * 512 : 4 * 512],
                )
                for _ in range(10):
                    t = dly.tile([128, 512], BF16, name="dlyt")
                    nc.vector.tensor_copy(t, zcur)
                    zcur = t
                nc.vector.tensor_add(
                    qt_sb[:, 0, 0:512], qt_sb[:, 0, 0:512], zcur
                )
                nc.vector.tensor_add(
                    qt_sb[:, 0, 512:1024], qt_sb[:, 0, 512:1024], zcur
                )

                def emit_d(st, engs):
                    """One phase-D s-tile: 8 output chunks x K=128 over the
                    two head-pairs."""
                    for ec in range(8):
                        pp = pps.tile([128, 512], F32, name="pp")
                        for hp in range(2):
                            nc.tensor.matmul(
                                pp,
                                lhsT=wo_sb[:, hp, ec * 128 : (ec + 1) * 128],
                                rhs=ctx2[:, hp, st * 512 : (st + 1) * 512],
                                start=(hp == 0),
                                stop=(hp == 1),
                            )
                        ob = pob.tile([128, 512], F32, name="ob")
                        engs[ec % len(engs)][0](ob, pp)
                        engs[ec % len(engs)][1](
                            out=outp[ec * 128 : (ec + 1) * 128,
                                     st * 512 : (st + 1) * 512],
                            in_=ob,
                        )

                dve_copy = lambda o, i: nc.vector.tensor_copy(o, i)
                act_copy = lambda o, i: nc.scalar.activation(
                    out=o, in_=i, func=AF.Copy
                )

                for blk, (pr, ih) in enumerate(
                    [(0, 0), (1, 0), (0, 1), (1, 1)]
                ):
                    for hh in range(2):
                        h = 2 * pr + hh
                        r0, r1 = hh * 64, (hh + 1) * 64
                        psav = [
                            avps.tile([DH + 1, 512], F32, name="psav")
                            for _ in range(2)
                        ]

                        # one-deep software pipeline: AV(jc) is emitted after
                        # QK(jc+1) so the PE always has QK work in flight
                        # while ScalarE computes exp
                        def emit_qk(jc):
                            psqk = qk2ps.tile([128, 1024], F32, name="psqk2")
                            for it in range(2):
                                i0 = ih * 1024 + it * 512
                                nc.tensor.matmul(
                                    psqk[:, it * 512 : (it + 1) * 512],
                                    lhsT=kt_sb[r0:r1, pr,
                                               jc * 128 : (jc + 1) * 128],
                                    rhs=qt_sb[r0:r1, pr, i0 : i0 + 512],
                                    start=True,
                                    stop=True,
                                )
                            return psqk

                        def emit_exp_av(psqk, jc):
                            ex = expp.tile([128, 1024], BF16, name="ex")
                            nc.scalar.activation(
                                out=ex, in_=psqk, func=AF.Exp, scale=SCALE
                            )
                            for it in range(2):
                                nc.tensor.matmul(
                                    psav[it],
                                    lhsT=v_sb[:, jc, h, :],
                                    rhs=ex[:, it * 512 : (it + 1) * 512],
                                    start=(jc == 0),
                                    stop=(jc == 15),
                                )

                        prev = emit_qk(0)
                        for jc in range(1, 16):
                            cur = emit_qk(jc)
                            emit_exp_av(prev, jc - 1)
                            prev = cur
                        emit_exp_av(prev, 15)

                        # per-pass normalization, fully off ScalarE and
                        # overlapped; its chain also paces the next pass's
                        # psav reuse, giving the HAM gate its idle window
                        for it in range(2):
                            i0 = ih * 1024 + it * 512
                            rsum = nrm.tile([1, 512], F32, name="rsum")
                            nc.vector.tensor_copy(
                                rsum, psav[it][DH : DH + 1, :]
                            )
                            rrow = nrm.tile([1, 512], F32, name="rrow")
                            nc.vector.reciprocal_approx_fast(
                                out=rrow, in_=rsum
                            )
                            rb = nrm.tile([64, 512], F32, name="rb")
                            nc.gpsimd.partition_broadcast(rb, rrow)
                            if hh == 0:
                                nc.vector.tensor_mul(
                                    ctx2[0:64, pr, i0 : i0 + 512],
                                    psav[it][0:DH, :],
                                    rb,
                                )
                            else:
                                # DVE lanes cannot shift partitions; land in
                                # a tmp and let a SBUF->SBUF DMA place head
                                # 2*pr+1 on partitions 64..127
                                ct = ctmp.tile([64, 512], BF16, name="ct")
                                nc.vector.tensor_mul(
                                    ct, psav[it][0:DH, :], rb
                                )
                                nc.sync.dma_start(
                                    out=ctx2[64:128, pr, i0 : i0 + 512],
                                    in_=ct,
                                )

                    # phase-D head: st0/st1 become ready once both ih=0
                    # blocks (blk 0 and 1) are normalized
                    if blk == 1:
                        emit_d(0, [(dve_copy, nc.sync.dma_start)])
                    elif blk == 2:
                        emit_d(1, [(dve_copy, nc.sync.dma_start)])

            # ---- Phase D tail: st2/st3 ----
            with (
                tc.tile_pool(name="pob2", bufs=4) as pob2,
                tc.tile_pool(name="pps2", bufs=4, space="PSUM") as pps2,
            ):
                for st in (2, 3):
                    for ec in range(8):
                        pp = pps2.tile([128, 512], F32, name="pp2")
                        for hp in range(2):
                            nc.tensor.matmul(
                                pp,
                                lhsT=wo_sb[:, hp, ec * 128 : (ec + 1) * 128],
                                rhs=ctx2[:, hp, st * 512 : (st + 1) * 512],
                                start=(hp == 0),
                                stop=(hp == 1),
                            )
                        ob = pob2.tile([128, 512], F32, name="ob2")
                        if ec % 2 == 0:
                            nc.vector.tensor_copy(ob, pp)
                            nc.sync.dma_start(
                                out=outp[ec * 128 : (ec + 1) * 128,
                                         st * 512 : (st + 1) * 512],
                                in_=ob,
                            )
                        else:
                            nc.scalar.activation(out=ob, in_=pp, func=AF.Copy)
                            nc.scalar.dma_start(
                                out=outp[ec * 128 : (ec + 1) * 128,
                                         st * 512 : (st + 1) * 512],
                                in_=ob,
                            )

    nc.compile()
    return nc


def _get_nc():
    global _CACHED_NC
    if _CACHED_NC is None:
        _CACHED_NC = build_nc()
    return _CACHED_NC


def make_in_maps(Q, K, V, W_q, b_q, W_k, b_k, W_v, b_v, W_o):
    import ml_dtypes

    BF = ml_dtypes.bfloat16
    xt = {}
    for b in range(B):
        xt["q", b] = np.ascontiguousarray(np.asarray(Q[b], np.float32).T).astype(BF)
        xt["k", b] = np.ascontiguousarray(np.asarray(K[b], np.float32).T).astype(BF)
        xt["v", b] = np.ascontiguousarray(np.asarray(V[b], np.float32).T).astype(BF)
    in_maps = []
    for c in range(NCORES):
        b, g = divmod(c, 4)
        L = slice(g * LOC, (g + 1) * LOC)
        wqt = np.ascontiguousarray(np.asarray(W_q, np.float32)[L, :].T).astype(BF)
        wkt = np.ascontiguousarray(np.asarray(W_k, np.float32)[L, :].T).astype(BF)
        wvt = np.ascontiguousarray(np.asarray(W_v, np.float32)[L, :].T).astype(BF)
        # head-pair-stacked W_o^T: wot[p, hp, e] = W_o[e, g*256 + hp*128 + p]
        wot = np.ascontiguousarray(
            np.asarray(W_o, np.float32)[:, L].T.reshape(2, 128, D)
            .transpose(1, 0, 2).astype(BF)
        )
        bqh = np.ascontiguousarray(np.asarray(b_q, np.float32)[L].reshape(2, 128).T)
        bkh = np.ascontiguousarray(np.asarray(b_k, np.float32)[L].reshape(2, 128).T)
        bvh = np.ascontiguousarray(
            np.broadcast_to(np.asarray(b_v, np.float32)[L], (128, LOC))
        )
        in_maps.append(
            dict(
                qt=xt["q", b], kt=xt["k", b], vt=xt["v", b],
                wqt=wqt, wkt=wkt, wvt=wvt, wot=wot,
                bq=bqh, bk=bkh, bv=bvh,
            )
        )
    return in_maps


def gather(results, b_o):
    out = np.zeros((B, S, D), dtype=np.float32)
    for c in range(NCORES):
        b = c // 4
        out[b] += results[c]["outp"].T
    out += np.asarray(b_o, np.float32)
    return out


def kernel(Q, K, V, W_q, b_q, W_k, b_k, W_v, b_v, W_o, b_o):
    nc = _get_nc()
    in_maps = make_in_maps(Q, K, V, W_q, b_q, W_k, b_k, W_v, b_v, W_o)
    res = run_bass_kernel_spmd(nc, in_maps, core_ids=list(range(NCORES)))
    return gather(res.results, b_o)


# revision 24
# speedup vs baseline: 1.2043x; 1.0135x over previous
"""Multi-head attention (B=2, S=2048, D=1024, H=16) on 8 Trainium2 cores.

Sharding: core c handles (batch b = c//4, head-group g = c%4 of 4 heads).
Megatron-style: W_q/k/v rows (output dims) column-sharded per head-group;
W_o columns row-sharded; the all-reduce over head-groups happens on the host
at gather time (sum of 4 partial projections per batch), where b_o is added.

All device data is bf16 (fp32 PSUM accumulation).  Inputs X^T are staged
whole-row ([128, 2048] = 256 KB contiguous DMAs) into SBUF ahead of the
compute that reads them, so no phase is DMA-bound.

The PE HAM activity governor on this part throttles the PE clock to 1.2 GHz
(K=4/8) under sustained near-100% PE duty, granting at most ~17us of 2.4 GHz
at a time; re-promotion needs idle in the window mix, and long saturated
stretches can lock the gate cold for 50-300us.  The kernel therefore keeps a
burst/idle rhythm the gate tolerates: in phase C, avps bufs=2 makes each
pass's first A@V wait on the previous pass's normalization chain (~2-4us of
genuine array idle per ~20us pass, nearly free because the ScalarE exp
stream is the phase bottleneck), and the B->C boundary has an explicit rest:
a serial DVE chain rooted at B's last output computes exact zeros that are
added in-place to the qt_sb region C's first matmuls read (a true data
dependency the list scheduler cannot hoist reads around).

Device layout (per core):
  Phase A: v projection (seq on partitions), v stored [128, 16 s-chunks,
  4 heads, 65] with a ones column per head so A@V accumulates softmax
  row-sums in PSUM row 64.
  Phase B: q/k projections transposed: q^T,k^T [256, 2048] as [128, 2, S].
  Phase C: per (pr, ih, hh) pass: scores^T [j, i] via K=64 matmuls (row
  pairs via base partitions 0/64), exp on ScalarE straight out of PSUM
  (scale=1/8, no max subtraction: scores ~ N(0,1)), ones-augmented A@V.
  ScalarE does nothing but exp; per-pass normalization: DVE rowsum copy ->
  DVE reciprocal_approx_fast -> gpsimd partition broadcast -> fused DVE
  (PSUM * rinv) -> bf16 ctx write (hh=1 lands via a partition-shifting
  SBUF->SBUF DMA so both heads of a pair stack on 128 partitions).
  Phase D: output projection with head-pair-stacked ctx [128, 2, S] and
  W_o^T [128, 2, 1024]: K=128 matmuls, half the instructions of a K=64
  layout.  st0/st1 tiles are emitted right after their inputs complete and
  overlap the last two attention blocks; only st2/st3 (~7us) trail.
"""

import numpy as np
from contextlib import ExitStack

import concourse.bass as bass
import concourse.bacc as bacc
import concourse.tile as tile
from concourse import mybir
from concourse.bass_utils import run_bass_kernel_spmd

F32 = mybir.dt.float32
BF16 = mybir.dt.bfloat16
AF = mybir.ActivationFunctionType

B, S, D = 2, 2048, 1024
H, DH = 16, 64
NCORES = 8
LOC = D // 4          # 256 local dims per head-group
SCALE = 1.0 / np.sqrt(DH)

_CACHED_NC = None


def build_nc():
    nc = bacc.Bacc("TRN2", target_bir_lowering=False, debug=False)

    qt = nc.dram_tensor("qt", [D, S], BF16, kind="ExternalInput").ap()
    kt = nc.dram_tensor("kt", [D, S], BF16, kind="ExternalInput").ap()
    vt = nc.dram_tensor("vt", [D, S], BF16, kind="ExternalInput").ap()
    wqt = nc.dram_tensor("wqt", [D, LOC], BF16, kind="ExternalInput").ap()
    wkt = nc.dram_tensor("wkt", [D, LOC], BF16, kind="ExternalInput").ap()
    wvt = nc.dram_tensor("wvt", [D, LOC], BF16, kind="ExternalInput").ap()
    wot = nc.dram_tensor("wot", [128, 2, D], BF16, kind="ExternalInput").ap()
    bq = nc.dram_tensor("bq", [128, 2], F32, kind="ExternalInput").ap()
    bk = nc.dram_tensor("bk", [128, 2], F32, kind="ExternalInput").ap()
    bv = nc.dram_tensor("bv", [128, LOC], F32, kind="ExternalInput").ap()
    outp = nc.dram_tensor("outp", [D, S], F32, kind="ExternalOutput").ap()

    with tile.TileContext(nc) as tc:
        with ExitStack() as ctx:
            wsb = ctx.enter_context(tc.tile_pool(name="wsb", bufs=1))
            big = ctx.enter_context(tc.tile_pool(name="big", bufs=1))

            # persistent SBUF state
            qt_sb = big.tile([128, 2, S], BF16, name="qt_sb")
            kt_sb = big.tile([128, 2, S], BF16, name="kt_sb")
            v_sb = big.tile([128, 16, 4, DH + 1], BF16, name="v_sb")
            # head-pair-stacked context: [0:64] = head 2*pr, [64:128] =
            # head 2*pr+1, so phase D contracts K=128 per head-pair
            ctx2 = big.tile([128, 2, S], BF16, name="ctx2")

            wq_sb = wsb.tile([128, 8, LOC], BF16, name="wq_sb")
            wk_sb = wsb.tile([128, 8, LOC], BF16, name="wk_sb")
            wv_sb = wsb.tile([128, 8, LOC], BF16, name="wv_sb")
            wo_sb = wsb.tile([128, 2, D], BF16, name="wo_sb")
            bq_sb = wsb.tile([128, 2], F32, name="bq_sb")
            bk_sb = wsb.tile([128, 2], F32, name="bk_sb")
            bv_sb = wsb.tile([128, LOC], F32, name="bv_sb")
            wup = wsb.tile([64, 128], BF16, name="wup")

            nc.gpsimd.memset(wup, 0.0)
            # ones column of v (accumulates softmax row-sums in A@V)
            nc.gpsimd.memset(v_sb[:, :, :, DH : DH + 1], 1.0)

            with ExitStack() as stage_ctx:
                stage = stage_ctx.enter_context(
                    tc.tile_pool(name="stage", bufs=1)
                )
                # whole-row input staging, both HW queues (~160 GB/s each):
                # vt first (phase A), then q/k weights, then qt/kt ahead of
                # phase B's reads
                vt_st = stage.tile([128, 8, S], BF16, name="vt_st")
                qt_st = stage.tile([128, 8, S], BF16, name="qt_st")
                kt_st = stage.tile([128, 8, S], BF16, name="kt_st")

                # fused DMAs (one issue instruction each) -- dma_start
                # issue costs ~0.6us of queue time, so fewer, fatter
                # transfers get the staging done well before phase B
                def _vt2(i, eng):
                    eng.dma_start(
                        out=vt_st[:, 2 * i : 2 * i + 2, :],
                        in_=vt[i * 256 : (i + 1) * 256, :].rearrange(
                            "(a p) s -> p a s", p=128
                        ),
                    )

                _vt2(0, nc.sync)
                _vt2(1, nc.scalar)
                nc.sync.dma_start(
                    out=wv_sb, in_=wvt.rearrange("(a p) r -> p a r", p=128)
                )
                nc.scalar.dma_start(
                    out=wq_sb, in_=wqt.rearrange("(a p) r -> p a r", p=128)
                )
                _vt2(2, nc.sync)
                _vt2(3, nc.scalar)
                nc.scalar.dma_start(
                    out=wk_sb, in_=wkt.rearrange("(a p) r -> p a r", p=128)
                )
                nc.scalar.dma_start(out=bq_sb, in_=bq)
                nc.scalar.dma_start(out=bk_sb, in_=bk)
                nc.sync.dma_start(out=bv_sb, in_=bv)
                nc.sync.dma_start(out=wo_sb, in_=wot)
                for half in range(2):
                    nc.sync.dma_start(
                        out=kt_st[:, 4 * half : 4 * half + 4, :],
                        in_=kt[half * 512 : (half + 1) * 512, :].rearrange(
                            "(a p) s -> p a s", p=128
                        ),
                    )
                    nc.scalar.dma_start(
                        out=qt_st[:, 4 * half : 4 * half + 4, :],
                        in_=qt[half * 512 : (half + 1) * 512, :].rearrange(
                            "(a p) s -> p a s", p=128
                        ),
                    )

                # ---- Warmup: dummy matmuls while input DMAs land ----
                with tc.tile_pool(name="wps", bufs=1, space="PSUM") as wps:
                    wp = wps.tile([64, 128], F32, name="wp")
                    for _ in range(36):
                        nc.tensor.matmul(
                            wp, lhsT=wup[:, 0:64], rhs=wup,
                            start=True, stop=True,
                        )

                # ---- Phase A: v projection (s on partitions) ----
                bv3 = bv_sb.rearrange("p (h d) -> p h d", h=4)
                with tc.tile_pool(name="vps", bufs=8, space="PSUM") as vps:
                    for sg in range(4):  # groups of 4 s-chunks of 128
                        psv = [
                            vps.tile([128, LOC], F32, name="psv")
                            for _ in range(4)
                        ]
                        for ds in range(8):
                            for c in range(4):
                                s0 = sg * 512 + c * 128
                                nc.tensor.matmul(
                                    psv[c],
                                    lhsT=vt_st[:, ds, s0 : s0 + 128],
                                    rhs=wv_sb[:, ds, :],
                                    start=(ds == 0),
                                    stop=(ds == 7),
                                )
                        for c in range(4):
                            sc = sg * 4 + c
                            nc.vector.tensor_add(
                                v_sb[:, sc, :, 0:DH],
                                psv[c].rearrange("p (h d) -> p h d", h=4),
                                bv3,
                            )

                # ---- Phase B: q/k projections (local dims on partitions) ----
                with tc.tile_pool(name="qkps", bufs=8, space="PSUM") as qkps:
                    for st in range(4):  # s-tiles of 512
                        ps = {}
                        for t in range(2):
                            for pr in range(2):
                                ps[t, pr] = qkps.tile(
                                    [128, 512], F32, name="psqk"
                                )
                        for ds in range(8):
                            for pr in range(2):
                                nc.tensor.matmul(
                                    ps[0, pr],
                                    lhsT=wq_sb[:, ds,
                                               pr * 128 : (pr + 1) * 128],
                                    rhs=qt_st[:, ds,
                                              st * 512 : (st + 1) * 512],
                                    start=(ds == 0),
                                    stop=(ds == 7),
                                )
                                nc.tensor.matmul(
                                    ps[1, pr],
                                    lhsT=wk_sb[:, ds,
                                               pr * 128 : (pr + 1) * 128],
                                    rhs=kt_st[:, ds,
                                              st * 512 : (st + 1) * 512],
                                    start=(ds == 0),
                                    stop=(ds == 7),
                                )
                        # PSUM -> SBUF bf16 with per-partition bias, off ACT
                        for pr in range(2):
                            nc.vector.tensor_scalar_add(
                                qt_sb[:, pr, st * 512 : (st + 1) * 512],
                                ps[0, pr],
                                bq_sb[:, pr : pr + 1],
                            )
                            nc.scalar.activation(
                                out=kt_sb[:, pr, st * 512 : (st + 1) * 512],
                                in_=ps[1, pr],
                                func=AF.Identity,
                                bias=bk_sb[:, pr : pr + 1],
                                scale=1.0,
                            )

            # ---- Phase C: attention + interleaved phase-D head ----
            with (
                tc.tile_pool(name="expp", bufs=4) as expp,
                tc.tile_pool(name="qk2ps", bufs=2, space="PSUM") as qk2ps,
                tc.tile_pool(name="avps", bufs=2, space="PSUM") as avps,
                tc.tile_pool(name="nrm", bufs=4) as nrm,
                tc.tile_pool(name="dly", bufs=2) as dly,
                tc.tile_pool(name="ctmp", bufs=4) as ctmp,
                tc.tile_pool(name="pps", bufs=2, space="PSUM") as pps,
                tc.tile_pool(name="pob", bufs=4) as pob,
            ):
                # B->C rest: ~4us PE-array idle as a true data dependency
                # (serial DVE chain -> exact zeros added in-place to the
                # qt_sb region C's first matmuls read)
                zcur = dly.tile([128, 512], BF16, name="dlyt")
                nc.vector.tensor_sub(
                    zcur,
                    kt_sb[:, 1, 3 * 512 : 4 * 512],
                    kt_sb[:, 1, 3 * 512 : 4 * 512],
                )
                for _ in range(10):
                    t = dly.tile([128, 512], BF16, name="dlyt")
                    nc.vector.tensor_copy(t, zcur)
                    zcur = t
                nc.vector.tensor_add(
                    qt_sb[:, 0, 0:512], qt_sb[:, 0, 0:512], zcur
                )
                nc.vector.tensor_add(
                    qt_sb[:, 0, 512:1024], qt_sb[:, 0, 512:1024], zcur
                )

                def emit_d(st, engs):
                    """One phase-D s-tile: 8 output chunks x K=128 over the
                    two head-pairs."""
                    for ec in range(8):
                        pp = pps.tile([128, 512], F32, name="pp")
                        for hp in range(2):
                            nc.tensor.matmul(
                                pp,
                                lhsT=wo_sb[:, hp, ec * 128 : (ec + 1) * 128],
                                rhs=ctx2[:, hp, st * 512 : (st + 1) * 512],
                                start=(hp == 0),
                                stop=(hp == 1),
                            )
                        ob = pob.tile([128, 512], F32, name="ob")
                        engs[ec % len(engs)][0](ob, pp)
                        engs[ec % len(engs)][1](
                            out=outp[ec * 128 : (ec + 1) * 128,
                                     st * 512 : (st + 1) * 512],
                            in_=ob,
                        )

                dve_copy = lambda o, i: nc.vector.tensor_copy(o, i)
                act_copy = lambda o, i: nc.scalar.activation(
                    out=o, in_=i, func=AF.Copy
                )

                for blk, (pr, ih) in enumerate(
                    [(0, 0), (1, 0), (0, 1), (1, 1)]
                ):
                    for hh in range(2):
                        h = 2 * pr + hh
                        r0, r1 = hh * 64, (hh + 1) * 64
                        psav = [
                            avps.tile([DH + 1, 512], F32, name="psav")
                            for _ in range(2)
                        ]

                        # one-deep software pipeline: AV(jc) is emitted after
                        # QK(jc+1) so the PE always has QK work in flight
                        # while ScalarE computes exp
                        def emit_qk(jc):
                            psqk = qk2ps.tile([128, 1024], F32, name="psqk2")
                            for it in range(2):
                                i0 = ih * 1024 + it * 512
                                nc.tensor.matmul(
                                    psqk[:, it * 512 : (it + 1) * 512],
                                    lhsT=kt_sb[r0:r1, pr,
                                               jc * 128 : (jc + 1) * 128],
                                    rhs=qt_sb[r0:r1, pr, i0 : i0 + 512],
                                    start=True,
                                    stop=True,
                                )
                            return psqk

                        def emit_exp_av(psqk, jc):
                            ex = expp.tile([128, 1024], BF16, name="ex")
                            nc.scalar.activation(
                                out=ex, in_=psqk, func=AF.Exp, scale=SCALE
                            )
                            for it in range(2):
                                nc.tensor.matmul(
                                    psav[it],
                                    lhsT=v_sb[:, jc, h, :],
                                    rhs=ex[:, it * 512 : (it + 1) * 512],
                                    start=(jc == 0),
                                    stop=(jc == 15),
                                )

                        prev = emit_qk(0)
                        for jc in range(1, 16):
                            cur = emit_qk(jc)
                            emit_exp_av(prev, jc - 1)
                            prev = cur
                        emit_exp_av(prev, 15)

                        # per-pass normalization, fully off ScalarE and
                        # overlapped; its chain also paces the next pass's
                        # psav reuse, giving the HAM gate its idle window
                        for it in range(2):
                            i0 = ih * 1024 + it * 512
                            rsum = nrm.tile([1, 512], F32, name="rsum")
                            nc.vector.tensor_copy(
                                rsum, psav[it][DH : DH + 1, :]
                            )
                            rrow = nrm.tile([1, 512], F32, name="rrow")
                            nc.vector.reciprocal_approx_fast(
                                out=rrow, in_=rsum
                            )
                            rb = nrm.tile([64, 512], F32, name="rb")
                            nc.gpsimd.partition_broadcast(rb, rrow)
                            if hh == 0:
                                nc.vector.tensor_mul(
                                    ctx2[0:64, pr, i0 : i0 + 512],
                                    psav[it][0:DH, :],
                                    rb,
                                )
                            else:
                                # DVE lanes cannot shift partitions; land in
                                # a tmp and let a SBUF->SBUF DMA place head
                                # 2*pr+1 on partitions 64..127
                                ct = ctmp.tile([64, 512], BF16, name="ct")
                                nc.vector.tensor_mul(
                                    ct, psav[it][0:DH, :], rb
                                )
                                nc.sync.dma_start(
                                    out=ctx2[64:128, pr, i0 : i0 + 512],
                                    in_=ct,
                                )

                    # phase-D head: st0/st1 become ready once both ih=0
                    # blocks (blk 0 and 1) are normalized
                    if blk == 1:
                        emit_d(0, [(dve_copy, nc.sync.dma_start)])
                    elif blk == 2:
                        emit_d(1, [(dve_copy, nc.sync.dma_start)])

            # ---- Phase D tail: st2/st3 ----
            with (
                tc.tile_pool(name="pob2", bufs=4) as pob2,
                tc.tile_pool(name="pps2", bufs=4, space="PSUM") as pps2,
            ):
                for st in (2, 3):
                    for ec in range(8):
                        pp = pps2.tile([128, 512], F32, name="pp2")
                        for hp in range(2):
                            nc.tensor.matmul(
                                pp,
                                lhsT=wo_sb[:, hp, ec * 128 : (ec + 1) * 128],
                                rhs=ctx2[:, hp, st * 512 : (st + 1) * 512],
                                start=(hp == 0),
                                stop=(hp == 1),
                            )
                        ob = pob2.tile([128, 512], F32, name="ob2")
                        if ec % 2 == 0:
                            nc.vector.tensor_copy(ob, pp)
                            nc.sync.dma_start(
                                out=outp[ec * 128 : (ec + 1) * 128,
                                         st * 512 : (st + 1) * 512],
                                in_=ob,
                            )
                        else:
                            nc.scalar.activation(out=ob, in_=pp, func=AF.Copy)
                            nc.scalar.dma_start(
                                out=outp[ec * 128 : (ec + 1) * 128,
                                         st * 512 : (st + 1) * 512],
                                in_=ob,
                            )

    nc.compile()
    return nc


def _get_nc():
    global _CACHED_NC
    if _CACHED_NC is None:
        _CACHED_NC = build_nc()
    return _CACHED_NC


def make_in_maps(Q, K, V, W_q, b_q, W_k, b_k, W_v, b_v, W_o):
    import ml_dtypes

    BF = ml_dtypes.bfloat16
    xt = {}
    for b in range(B):
        xt["q", b] = np.ascontiguousarray(np.asarray(Q[b], np.float32).T).astype(BF)
        xt["k", b] = np.ascontiguousarray(np.asarray(K[b], np.float32).T).astype(BF)
        xt["v", b] = np.ascontiguousarray(np.asarray(V[b], np.float32).T).astype(BF)
    in_maps = []
    for c in range(NCORES):
        b, g = divmod(c, 4)
        L = slice(g * LOC, (g + 1) * LOC)
        wqt = np.ascontiguousarray(np.asarray(W_q, np.float32)[L, :].T).astype(BF)
        wkt = np.ascontiguousarray(np.asarray(W_k, np.float32)[L, :].T).astype(BF)
        wvt = np.ascontiguousarray(np.asarray(W_v, np.float32)[L, :].T).astype(BF)
        # head-pair-stacked W_o^T: wot[p, hp, e] = W_o[e, g*256 + hp*128 + p]
        wot = np.ascontiguousarray(
            np.asarray(W_o, np.float32)[:, L].T.reshape(2, 128, D)
            .transpose(1, 0, 2).astype(BF)
        )
        bqh = np.ascontiguousarray(np.asarray(b_q, np.float32)[L].reshape(2, 128).T)
        bkh = np.ascontiguousarray(np.asarray(b_k, np.float32)[L].reshape(2, 128).T)
        bvh = np.ascontiguousarray(
            np.broadcast_to(np.asarray(b_v, np.float32)[L], (128, LOC))
        )
        in_maps.append(
            dict(
                qt=xt["q", b], kt=xt["k", b], vt=xt["v", b],
                wqt=wqt, wkt=wkt, wvt=wvt, wot=wot,
                bq=bqh, bk=bkh, bv=bvh,
            )
        )
    return in_maps


def gather(results, b_o):
    out = np.zeros((B, S, D), dtype=np.float32)
    for c in range(NCORES):
        b = c // 4
        out[b] += results[c]["outp"].T
    out += np.asarray(b_o, np.float32)
    return out


def kernel(Q, K, V, W_q, b_q, W_k, b_k, W_v, b_v, W_o, b_o):
    nc = _get_nc()
    in_maps = make_in_maps(Q, K, V, W_q, b_q, W_k, b_k, W_v, b_v, W_o)
    res = run_bass_kernel_spmd(nc, in_maps, core_ids=list(range(NCORES)))
    return gather(res.results, b_o)
